# revision 60
# baseline (speedup 1.0000x reference)
"""Trainium2 Bass kernel for AttentionPooling (ragged span attention pooling).

Math restructuring (vs the reference's gather-then-project):
  - K/V projections are computed once per unique token, not per gathered span
    token.
  - The query is a single shared vector, so per-span softmax factorizes:
        attn[s,n,l] = e[start_s+l, n] / Z[s,n],   e[t,n] = exp(q_n . k_{t,n} / 8)
        Z[s,n]      = sum_{t in span_s} e[t,n]
    Hence  ctx[s] = (1/Z[s]) * sum_t W[t,s] * (e[t] (x) V[t])  with the SAME 0/1
    banded window matrix W for all heads -> one dense matmul per core.
  - Everything runs feature-major (feature dim on partitions, spans on the free
    dim) end-to-end; the final output is written feature-major [H, C] and the
    host transposes (free).
  - Masked-out spans are dropped on the host (span compaction) AND spans are
    sorted by start so each core covers a narrow token window: V'/ctx/window
    work scales with the window (TCL*128 tokens), not the full T=512.
  - V'-bias (bv) is folded through the out-projection into the LN1 bias.
  - LayerNorms use centered-weight tricks (wout/W2 columns centered on the
    host) so no on-device mean subtraction is needed (gamma=1/beta=0 path).
  - Attention path in fp32r/bf16; FFN in fp8e4 (DoubleRow, 2 K-chunks per
    matmul) with fp32 PSUM accumulation, weights pre-scaled by 16/32 on the
    host (scales folded into the relu and the LN2 scale-invariance).

Sharding: unmasked spans of batch 0 (sorted by start) split over cores 0-3,
batch 1 over cores 4-7. Weights replicated.
"""

import sys
import numpy as np

if "/opt/trn_rl_repo" not in sys.path:
    sys.path.insert(0, "/opt/trn_rl_repo")

B, T, S, H, L, NH = 2, 512, 2048, 768, 32, 12
HD = H // NH            # 64
INTERMED = 4 * H        # 3072
NCORES = 8
HC = H // 128            # 6 feature chunks
IC = INTERMED // 128     # 24 intermediate chunks
VW = H + NH              # 780: [e-scaled V | e]
WG = 4                   # i-chunks per streamed weight group
NG = IC // WG            # 6 weight groups
NP = IC // 2             # 12 i-chunk pairs (fp8 DoubleRow)
EPS = 1e-5

FP8 = False              # fp8e4 DoubleRow FFN (measured: rel err 5e-2 AND
                         # DoubleRow MMs run at 2N cycles here -> no win)
S1 = 16.0                # host scale on W1 (fp8 range); undone in the relu
S2 = 32.0                # host scale on W2; folded via LN2 scale-invariance

_COMPILED = {}


def _build(C, TCL, gb_identity=False, fp8=FP8):
    """C = spans per core, TCL = 128-token chunks per core (compile-time)."""
    import concourse.bacc as bacc
    import concourse.tile as tile
    from concourse import mybir
    from concourse.alu_op_type import AluOpType as Op

    f32 = mybir.dt.float32
    f32r = mybir.dt.float32r
    bf16 = mybir.dt.bfloat16
    fp8e4 = mybir.dt.float8e4
    Act = mybir.ActivationFunctionType
    DR = mybir.MatmulPerfMode.DoubleRow
    mm_dt = fp8e4 if fp8 else bf16

    nc = bacc.Bacc("TRN2", target_bir_lowering=False, debug=False, num_devices=NCORES)

    def din(name, shape, dt=f32):
        return nc.dram_tensor(name, list(shape), dt, kind="ExternalInput").ap()

    TW = TCL * 128                    # token window size
    xT = din("xT", [H, TW], bf16)     # (x = token_reps + pe), transposed window
    wvl = din("wvl", [H, VW], bf16)   # [Wv.T | wq2.T]
    bvl = din("bvl", [1, VW], bf16)   # [0 | q.bk per head] (bv folded into b1c)
    starts = din("starts", [1, C])    # span starts (f32, global token ids)
    ends = din("ends", [1, C])        # span start + len (f32)
    iot = din("iota", [128, TCL])     # t_global per (partition, t-chunk)
    ssel = din("ssel", [NH, H])       # head selector: ssel[n,h'] = (h'//64==n)
    wout = din("wout", [H, H], bf16)  # (Wout - colmean(Wout)).T  [h', h]
    b1c = din("b1c", [H])             # centered (out_b + bv@WoutC.T + query)
    gco = din("gco", [H])             # norm gamma
    bco = din("bco", [H])             # norm beta
    if fp8:
        w1t = din("w1t", [NG, 128, HC // 2, 2, WG * 128], fp8e4)
        w2t = din("w2t", [NG, 128, WG // 2, 2, H], fp8e4)
    else:
        w1t = din("w1t", [NG, 128, HC, WG * 128], bf16)
        w2t = din("w2t", [NG, 128, WG, H], bf16)
    b1r = din("b1r", [INTERMED])      # ffn_b1
    b2c = din("b2c", [H])             # centered ffn_b2, pre-scaled by S2
    onesv = din("onesv", [128])       # ones (f32r matmul operand source)
    onesb = din("onesb", [128], bf16)  # ones (bf16)


    out = nc.dram_tensor("out", [H, C], bf16, kind="ExternalOutput").ap()

    with tile.TileContext(nc) as tc:
        with (
            tc.tile_pool(name="consts", bufs=1) as cp,
            tc.tile_pool(name="x1keep", bufs=1) as x1p,
            tc.tile_pool(name="w1s", bufs=3) as w1p,
            tc.tile_pool(name="w2s", bufs=6) as w2p,
        ):
            # ---- small consts; attention-critical ones FIRST (the gpsimd
            # SWDGE queue serializes ~1-2us per transfer, so queue position
            # is arrival time) ----
            ones1 = cp.tile([1, 128], f32r)      # K=1 matmul lhsT
            nc.gpsimd.dma_start(ones1[:], onesv.unsqueeze(0).bitcast(f32r))
            ssel_sb = cp.tile([NH, H], f32r)
            nc.gpsimd.dma_start(ssel_sb[:], ssel.bitcast(f32r))
            # starts/ends/iota ride the fast HWDGE queues ahead of the
            # weight streams (the SWDGE const queue lands too late for the
            # window-matrix build)
            starts_r = cp.tile([1, C], f32r)
            nc.sync.dma_start(starts_r[:], starts.bitcast(f32r))
            ends_r = cp.tile([1, C], f32r)
            nc.scalar.dma_start(ends_r[:], ends.bitcast(f32r))
            iota_sb = cp.tile([128, TCL], f32)
            nc.sync.dma_start(iota_sb[:], iot)
            ones1w = cp.tile([1, 512], bf16)     # warmup rhs
            nc.vector.memset(ones1w, 1.0)
            ones1b = cp.tile([1, 128], bf16)
            nc.gpsimd.dma_start(ones1b[:], onesb.unsqueeze(0))
            bvl_sb = cp.tile([1, VW], bf16)
            nc.gpsimd.dma_start(bvl_sb[:], bvl)
            eps1 = cp.tile([1, 1], f32)
            nc.vector.memset(eps1, EPS)
            onescol = cp.tile([128, 1], f32r)    # partition-colsum lhsT
            nc.gpsimd.dma_start(onescol[:], onesv.unsqueeze(1).bitcast(f32r))
            gcol = cp.tile([128, HC], f32)      # gamma as per-partition cols
            nc.gpsimd.dma_start(gcol[:], gco.rearrange("(c p) -> p c", p=128))
            bcol = cp.tile([128, HC], f32)
            nc.gpsimd.dma_start(bcol[:], bco.rearrange("(c p) -> p c", p=128))
            b1ccol = cp.tile([128, HC], f32)
            nc.gpsimd.dma_start(b1ccol[:], b1c.rearrange("(c p) -> p c", p=128))
            b1col = cp.tile([128, IC], f32)
            nc.gpsimd.dma_start(b1col[:], b1r.rearrange("(c p) -> p c", p=128))
            b2ccol = cp.tile([128, HC], f32)
            nc.gpsimd.dma_start(b2ccol[:], b2c.rearrange("(c p) -> p c", p=128))
            eps2s = cp.tile([1, 1], f32)    # LN2 eps (scaled in fp8 mode)
            nc.vector.memset(eps2s, EPS * (S2 * S2 if fp8 else 1.0))

            # x1 kept (up to) three ways: matmul operand, unit-scale residual
            # basis, S2-scaled residual for the (scaled-W2) FFN2 add.
            x1b = x1p.tile([128, HC, C], bf16, name="x1b")
            if fp8:
                x1mm = x1p.tile([128, HC, C], mm_dt, name="x1mm")
                x1s2 = x1p.tile([128, HC, C], bf16, name="x1s2")
            else:
                x1mm, x1s2 = x1b, x1b

            # FFN weight tiles allocated early (stable addresses); all of W2
            # goes on the gpsimd queue (idle during attention), W1 groups 0-2
            # trail the attention-critical loads on sync.
            if fp8:
                w1g = [w1p.tile([128, HC // 2, 2, WG * 128], fp8e4, tag="w1",
                                name=f"w1g{g}") for g in range(3)]
                w2g = [w2p.tile([128, WG // 2, 2, H], fp8e4, tag="w2",
                                name=f"w2g{g}") for g in range(NG)]
            else:
                w1g = [w1p.tile([128, HC, WG * 128], bf16, tag="w1",
                                name=f"w1g{g}") for g in range(3)]
                w2g = [w2p.tile([128, WG, H], bf16, tag="w2",
                                name=f"w2g{g}") for g in range(NG)]

            # ---------------- attention (feature-major) ----------------
            with (
                tc.tile_pool(name="attn", bufs=1) as ap_,
                tc.tile_pool(name="attn_s", bufs=2) as asml,
            ):
                xTc = [ap_.tile([128, TW], bf16, name=f"xTc{c}")
                       for c in range(HC)]
                wvlc = [ap_.tile([128, VW], bf16, name=f"wvlc{c}")
                        for c in range(HC)]
                xTr = xT.rearrange("(c p) t -> c p t", p=128)
                wvlr = wvl.rearrange("(c p) n -> c p n", p=128)
                # attention-critical loads first, round-robin on two queues
                qs = [nc.sync, nc.scalar]
                for c in range(HC):
                    qs[0].dma_start(wvlc[c][:], wvlr[c])
                    qs[1].dma_start(xTc[c][:], xTr[c])

                # separate tiles for e*V and e: keeps the Z matmul (reads e)
                # from false-serializing behind the e*V DVE multiply (Tile
                # tracks deps per tile)
                veV = ap_.tile([128, TCL, H], bf16)   # e*V, token-major
                veE = ap_.tile([128, TCL, NH], bf16)  # e,   token-major
                wt = ap_.tile([128, TCL, C], bf16)    # W[t, s] 0/1 window
                ctxN = ap_.tile([128, HC, C], bf16)   # normalized ctx
                ycs = ap_.tile([128, HC, C], bf16)    # out_proj + b1c

                with (
                    tc.tile_pool(name="psAux", bufs=1, space="PSUM") as psAux,
                    tc.tile_pool(name="psV", bufs=2, space="PSUM") as psV,
                ):
                    # warm up the PE (HAM clock gate) while the loads land;
                    # also preload the Exp activation table.
                    wf = asml.tile([1, 128], bf16, tag="wf")
                    nc.vector.memset(wf, 1.0)
                    dexp = asml.tile([1, 1], f32, tag="dexp")
                    nc.scalar.activation(dexp[:], eps1[:], Act.Exp)
                    dum = psAux.tile([128, 512], f32, tag="warm")
                    for k in range(8):
                        nc.tensor.matmul(dum[:], wf[:], ones1w[:],
                                         start=(k == 0), stop=(k == 7))
                    # broadcast span starts/ends across partitions on the PE
                    startsB = psAux.tile([128, C], f32, tag="sb",
                                         name="startsB")
                    nc.tensor.matmul(startsB[:], ones1[:], starts_r[:],
                                     start=True, stop=True)
                    endsB = psAux.tile([128, C], f32, tag="eb",
                                       name="endsB")
                    nc.tensor.matmul(endsB[:], ones1[:], ends_r[:],
                                     start=True, stop=True)
                    # remaining weights on the two HWDGE queues (keeping the
                    # gpsimd queue empty so PSUM pool releases aren't stuck
                    # behind slow SWDGE DMA waits)
                    wout_sb = ap_.tile([128, HC, H], bf16)
                    nc.sync.dma_start(wout_sb[:],
                                      wout.rearrange("(c p) n -> p c n", p=128))
                    for g in range(3):
                        nc.sync.dma_start(w1g[g][:], w1t[g])
                    for g in range(NG):
                        qs[g % 2].dma_start(w2g[g][:], w2t[g])

                    # W[t, s] = (start_s <= t) & (t < end_s)  (needs no x)
                    for t in range(TCL):
                        lt = asml.tile([128, C], f32, tag="uexp", name="lt")
                        nc.vector.tensor_scalar(
                            lt[:], endsB[:], iota_sb[:, t:t + 1], None,
                            Op.is_gt)
                        nc.vector.scalar_tensor_tensor(
                            wt[:, t, :], startsB[:], iota_sb[:, t:t + 1],
                            lt[:], Op.is_le, Op.mult)

                    # V' = x @ [Wv.T | wq2.T] (+ e-logit bias)  (token-major)
                    for t in range(TCL):
                        vp = psV.tile([128, VW], f32, tag="vp")
                        for lo, hi in ((0, 512), (512, VW)):
                            for c in range(HC):
                                nc.tensor.matmul(
                                    vp[:, lo:hi],
                                    xTc[c][:, t * 128:(t + 1) * 128],
                                    wvlc[c][:, lo:hi],
                                    start=(c == 0),
                                    stop=(c == HC - 1 and hi != VW),
                                )
                        nc.tensor.matmul(
                            vp[:, H:VW], ones1b[:], bvl_sb[:, H:VW],
                            start=False, stop=True,
                        )
                        # e = exp(logits)
                        nc.scalar.activation(veE[:, t, :], vp[:, H:VW],
                                             Act.Exp)
                        # veV[:, t] = V * e (per-head broadcast of e)
                        e_b = veE[:, t, :].unsqueeze(2).broadcast_to(
                            [128, NH, HD])
                        nc.vector.tensor_tensor(
                            veV[:, t, :].rearrange("p (n d) -> p n d", d=HD),
                            vp[:, 0:H].rearrange("p (n d) -> p n d", d=HD),
                            e_b, Op.mult,
                        )
                    # preload the Sqrt table now that Exp is done with ACT
                    # (read a ve value so this schedules AFTER the Exp ops)
                    dsq = asml.tile([1, 1], f32, tag="dexp", name="dsq")
                    nc.scalar.activation(dsq[:], veE[0:1, TCL - 1, 0:1],
                                         Act.Sqrt)

                with (
                    tc.tile_pool(name="psMM", bufs=6, space="PSUM") as psMM,
                    tc.tile_pool(name="psS", bufs=2, space="PSUM") as psS,
                ):
                    cps = [psMM.tile([128, C], f32, tag="mm512",
                                     name=f"cp{c}") for c in range(HC)]
                    # ctx t=0 first: the PE stays busy in-order while Z's
                    # bank may still be held by the (late-released) V' buffer
                    for c in range(HC):
                        nc.tensor.matmul(
                            cps[c][:], veV[:, 0, c * 128:(c + 1) * 128],
                            wt[:, 0, :], start=True, stop=(TCL == 1))
                    # Z[n, s] then u = 1/Z  (Z >= exp(tiny logit) ~ 1 always:
                    # every span keeps >= 1 in-window token; approx_fast is
                    # ~18 bits, ample for an attn scale kept in bf16 later)
                    zp = psS.tile([NH, C], f32, tag="small", name="zp")
                    for t in range(TCL):
                        nc.tensor.matmul(
                            zp[:], veE[:, t, :], wt[:, t, :],
                            start=(t == 0), stop=(t == TCL - 1))
                    uf = asml.tile([NH, C], f32, tag="uf", name="uf")
                    nc.vector.reciprocal_approx_fast(uf[:], zp[:])
                    u_sb = asml.tile([NH, C], f32r, tag="u")
                    with nc.allow_low_precision(reason="f32r is fp32-width"):
                        nc.vector.tensor_copy(u_sb[:], uf[:])

                    # remaining ctx accumulation + u broadcast/apply
                    uexps = []
                    for c in range(HC):
                        cp_ = cps[c]
                        for t in range(1, TCL):
                            nc.tensor.matmul(
                                cp_[:], veV[:, t, c * 128:(c + 1) * 128],
                                wt[:, t, :],
                                start=False, stop=(t == TCL - 1))
                        up = psS.tile([128, C], f32, tag="small",
                                      name=f"up{c}")
                        nc.tensor.matmul(
                            up[:], ssel_sb[:, c * 128:(c + 1) * 128],
                            u_sb[:], start=True, stop=True)
                        uexp = asml.tile([128, C], bf16, tag="uexp",
                                         name=f"uexp{c}")
                        nc.scalar.activation(uexp[:], up[:], Act.Identity)
                        uexps.append(uexp)
                        if c >= 1:
                            nc.vector.tensor_tensor(ctxN[:, c - 1, :],
                                                    cps[c - 1][:],
                                                    uexps[c - 1][:], Op.mult)
                    nc.vector.tensor_tensor(ctxN[:, HC - 1, :], cps[HC - 1][:],
                                            uexps[HC - 1][:], Op.mult)

                    # out_proj (centered weights) + LN1, feature-major
                    varp = psS.tile([1, C], f32, tag="small", name="varp")
                    sqs = []
                    for m in range(HC):
                        aop = psMM.tile([128, C], f32, tag="mm512",
                                        name=f"aop{m}")
                        for c in range(HC):
                            nc.tensor.matmul(
                                aop[:], wout_sb[:, c, m * 128:(m + 1) * 128],
                                ctxN[:, c, :],
                                start=(c == 0), stop=(c == HC - 1))
                        # yc = aop + centered bias (per-partition) on ACT
                        nc.scalar.activation(ycs[:, m, :], aop[:],
                                             Act.Identity,
                                             bias=b1ccol[:, m:m + 1])
                        sq = asml.tile([128, C], f32r, tag="sq",
                                       name=f"sq{m}")
                        with nc.allow_low_precision(reason="f32r fp32-width"):
                            nc.vector.tensor_tensor(sq[:], ycs[:, m, :],
                                                    ycs[:, m, :], Op.mult)
                        sqs.append(sq)
                        nc.tensor.matmul(
                            varp[:], onescol[:], sq[:],
                            start=(m == 0), stop=(m == HC - 1))
                    # keep the PE warm through the serial rstd chain (a >3.4us
                    # idle gap would re-throttle the PE clock to half rate);
                    # reading ycs[5] anchors these INTO the chain window --
                    # dependency-free matmuls would get scheduled early
                    wrm = psS.tile([128, C], f32, tag="small", name="wrm")
                    for k in range(10):
                        nc.tensor.matmul(wrm[:], ycs[:, HC - 1, 0:128],
                                         ycs[:, HC - 1, :],
                                         start=(k == 0), stop=(k == 9))
                    # rstd = 1/sqrt(varp/H + eps); approx_fast (~18 bits) is
                    # ample for a scale kept in bf16 downstream
                    sd = asml.tile([1, C], f32, tag="sd")
                    nc.scalar.activation(sd[:], varp[:], Act.Sqrt,
                                         bias=eps1[:], scale=1.0 / H)
                    rf = asml.tile([1, C], f32, tag="rf2", name="rf")
                    nc.vector.reciprocal_approx_fast(rf[:], sd[:])
                    rstd = asml.tile([1, C], bf16, tag="rstd")
                    with nc.allow_low_precision(reason="per-span scale; "
                                                "bf16 downstream"):
                        nc.vector.tensor_copy(rstd[:], rf[:])
                    rp = psS.tile([128, C], f32, tag="small", name="rp")
                    nc.tensor.matmul(rp[:], ones1b[:], rstd[:],
                                     start=True, stop=True)
                    # second warm group: bridges rp -> x1b[0] -> first FFN1 MM
                    # (anchored on rstd so it lands exactly in that window)
                    wrm_b = psS.tile([128, C], f32, tag="small", name="wrm_b")
                    for k in range(4):
                        nc.tensor.matmul(wrm_b[:], ones1b[:], rstd[:],
                                         start=(k == 0), stop=(k == 3))
                    rpb = asml.tile([128, C], bf16, tag="rpb")
                    nc.scalar.activation(rpb[:], rp[:], Act.Identity)
                    for m in range(HC):
                        if gb_identity:
                            # chunk 0 straight off the PSUM broadcast (rpb
                            # lags by one ACT op); rest via bf16 2x-mode
                            if m == 0:
                                nc.vector.tensor_tensor(x1b[:, m, :],
                                                        ycs[:, m, :],
                                                        rp[:], Op.mult)
                            else:
                                nc.vector.tensor_tensor(x1b[:, m, :],
                                                        ycs[:, m, :],
                                                        rpb[:], Op.mult)
                        else:
                            tmp = asml.tile([128, C], f32, tag="tmp",
                                            name="tmp")
                            nc.vector.tensor_tensor(tmp[:], ycs[:, m, :],
                                                    rp[:], Op.mult)
                            nc.scalar.activation(x1b[:, m, :], tmp[:],
                                                 Act.Identity,
                                                 scale=gcol[:, m:m + 1],
                                                 bias=bcol[:, m:m + 1])
                        if fp8:
                            # fp8 matmul operand (ACT) + S2-scaled residual
                            nc.scalar.activation(x1mm[:, m, :], x1b[:, m, :],
                                                 Act.Copy)
                            nc.vector.tensor_scalar(x1s2[:, m, :],
                                                    x1b[:, m, :], S2, None,
                                                    Op.mult)

            # ---------------- FFN + LN2 (feature-major, pipelined) --------
            with (
                tc.tile_pool(name="ffn", bufs=1) as fp_,
                tc.tile_pool(name="ffn_s", bufs=2) as fsml,
                tc.tile_pool(name="ffn_o", bufs=6) as fout,
                tc.tile_pool(name="psH", bufs=2, space="PSUM") as psH,
                tc.tile_pool(name="psF", bufs=1, space="PSUM") as psF,
            ):
                h1s = fp_.tile([128, IC, C], mm_dt)
                zF = fp_.tile([128, HC, C], bf16)
                h2T = [psF.tile([128, C], f32, tag=f"h2_{m}", name=f"h2T{m}")
                       for m in range(HC)]

                x1op = x1mm if fp8 else x1b

                def ffn2_pair(ip):
                    g, lp = divmod(ip, WG // 2)
                    for m in range(HC):
                        nc.tensor.matmul(
                            h2T[m][:],
                            w2g[g][:, lp, :, m * 128:(m + 1) * 128],
                            h1s[:, 2 * ip:2 * ip + 2, :],
                            start=(ip == 0), stop=False, perf_mode=DR)

                def ffn2_single(i):
                    g, l = divmod(i, WG)
                    for m in range(HC):
                        nc.tensor.matmul(
                            h2T[m][:],
                            w2g[g][:, l, m * 128:(m + 1) * 128],
                            h1s[:, i, :],
                            start=(i == 0), stop=False)

                NTAIL = 7        # bf16: last i-chunks run m-outer
                NTAILP = 4       # fp8: last i-chunk pairs run m-outer

                for i in range(IC):
                    g, l = divmod(i, WG)
                    if l == 0 and g >= 3:  # stream remaining W1 groups
                        if fp8:
                            w1g[g % 3] = w1p.tile(
                                [128, HC // 2, 2, WG * 128], fp8e4,
                                tag="w1", name=f"w1g{g}")
                        else:
                            w1g[g % 3] = w1p.tile(
                                [128, HC, WG * 128], bf16,
                                tag="w1", name=f"w1g{g}")
                        nc.sync.dma_start(w1g[g % 3][:], w1t[g])
                    w1 = w1g[g % 3]
                    h1p = psH.tile([128, C], f32, tag="h1p")
                    if fp8:
                        for cp2 in range(HC // 2):
                            nc.tensor.matmul(
                                h1p[:],
                                w1[:, cp2, :, l * 128:(l + 1) * 128],
                                x1op[:, 2 * cp2:2 * cp2 + 2, :],
                                start=(cp2 == 0), stop=(cp2 == HC // 2 - 1),
                                perf_mode=DR)
                    else:
                        for c in range(HC):
                            nc.tensor.matmul(
                                h1p[:], w1[:, c, l * 128:(l + 1) * 128],
                                x1op[:, c, :],
                                start=(c == 0), stop=(c == HC - 1))
                    nc.scalar.activation(h1s[:, i, :], h1p[:], Act.Relu,
                                         bias=b1col[:, i:i + 1],
                                         scale=1.0 / S1 if fp8 else 1.0)
                    if fp8:
                        if i % 2 == 1 and 1 <= i // 2 <= NP - NTAILP:
                            ffn2_pair(i // 2 - 1)
                    else:
                        if 1 <= i <= IC - NTAIL:
                            ffn2_single(i - 1)

                # LN2 (no mean: W2/b2 centered, x1 zero-mean) + output.
                # Last chunks run m-outer: h2T[m] finishes early so its
                # z/square/var chain overlaps the remaining matmuls.
                varp2 = psH.tile([1, C], f32, tag="h1p", name="varp2")
                sq2s = []
                for m in range(HC):
                    if fp8:
                        for ip in range(NP - NTAILP, NP):
                            g, lp = divmod(ip, WG // 2)
                            nc.tensor.matmul(
                                h2T[m][:],
                                w2g[g][:, lp, :, m * 128:(m + 1) * 128],
                                h1s[:, 2 * ip:2 * ip + 2, :],
                                start=False, stop=(ip == NP - 1),
                                perf_mode=DR)
                    else:
                        for i in range(IC - NTAIL, IC):
                            g, l = divmod(i, WG)
                            nc.tensor.matmul(
                                h2T[m][:],
                                w2g[g][:, l, m * 128:(m + 1) * 128],
                                h1s[:, i, :],
                                start=False, stop=(i == IC - 1))
                    # z = (h2 + S2*b2c) + S2*x1  (one DVE op, feature-major)
                    nc.vector.scalar_tensor_tensor(
                        zF[:, m, :], h2T[m][:], b2ccol[:, m:m + 1],
                        x1s2[:, m, :], Op.add, Op.add)
                    sq2 = fsml.tile([128, C], f32r, tag="sq2", name=f"sq2{m}")
                    with nc.allow_low_precision(reason="f32r is fp32-width"):
                        nc.scalar.activation(sq2[:], zF[:, m, :], Act.Square)
                    sq2s.append(sq2)
                    if m >= 2:
                        nc.tensor.matmul(
                            varp2[:], onescol[:], sq2s[m - 2][:],
                            start=(m - 2 == 0), stop=False)
                for m in (HC - 2, HC - 1):
                    nc.tensor.matmul(
                        varp2[:], onescol[:], sq2s[m][:],
                        start=False, stop=(m == HC - 1))
                wrm2 = psH.tile([128, C], f32, tag="h1p", name="wrm2")
                for k in range(10):
                    nc.tensor.matmul(wrm2[:], zF[:, HC - 1, 0:128],
                                     zF[:, HC - 1, :],
                                     start=(k == 0), stop=(k == 9))
                sd2 = fsml.tile([1, C], f32, tag="sd2")
                nc.scalar.activation(sd2[:], varp2[:], Act.Sqrt,
                                     bias=eps2s[:], scale=1.0 / H)
                rf2 = fsml.tile([1, C], f32, tag="rf2b", name="rf2")
                nc.vector.reciprocal_approx_fast(rf2[:], sd2[:])
                rstd2 = fsml.tile([1, C], bf16, tag="rstd2")
                with nc.allow_low_precision(reason="output is bf16 anyway"):
                    nc.vector.tensor_copy(rstd2[:], rf2[:])
                rp2 = psH.tile([128, C], f32, tag="h1p", name="rp2")
                nc.tensor.matmul(rp2[:], ones1b[:], rstd2[:],
                                 start=True, stop=True)
                rp2b = fsml.tile([128, C], bf16, tag="rp2b")
                nc.scalar.activation(rp2b[:], rp2[:], Act.Identity)
                oqs = [nc.sync, nc.scalar, nc.gpsimd]
                for m in range(HC):
                    o = fout.tile([128, C], bf16, tag="o", name=f"o{m}")
                    if gb_identity:
                        # chunk 0 straight off the PSUM broadcast (rp2b lags
                        # by one ACT op); rest via bf16 2x-mode
                        nc.vector.tensor_tensor(o[:], zF[:, m, :],
                                                rp2[:] if m == 0 else rp2b[:],
                                                Op.mult)
                    else:
                        on = fout.tile([128, C], f32, tag="on", name=f"on{m}")
                        nc.vector.tensor_tensor(on[:], zF[:, m, :],
                                                rp2b[:], Op.mult)
                        nc.scalar.activation(o[:], on[:], Act.Identity,
                                             scale=gcol[:, m:m + 1],
                                             bias=bcol[:, m:m + 1])
                    oqs[m % 2].dma_start(out[m * 128:(m + 1) * 128, :], o[:])
    nc.compile()
    return nc


def _plan(span_masks, span_ids):
    """Compact unmasked spans, dedup identical (start,end) pairs, sort by
    start, split each batch's unique spans across its 4 cores (so each core
    covers a narrow token window). Returns (per_core_idx, C, scatter) where
    per_core_idx holds representative original span indices and scatter maps
    every unmasked original span to (its batch's unique-span position)."""
    masks = np.asarray(span_masks)
    sid = np.asarray(span_ids)
    per_core_idx, scatter = [], []
    cpb = NCORES // B
    for b in range(B):
        idx = np.nonzero(masks[b])[0]
        key = sid[b, idx, 0] * 1024 + sid[b, idx, 1]   # start-major sort key
        uk, first, inv = np.unique(key, return_index=True,
                                   return_inverse=True)
        rep = idx[first]                               # one original per uniq
        per_core_idx.extend(np.array_split(rep, cpb))
        scatter.append((idx, inv))
    cmax = max(len(ix) for ix in per_core_idx)
    C = (cmax + 7) // 8 * 8
    C = min(max(C, 128), 512)
    return per_core_idx, C, scatter


def _host_prepare(inputs, per_core_idx, C):
    """Host-side packing: tiny index/weight reshapes, no heavy math.
    Returns (in_maps, TCL, sorted per_core_idx)."""
    import ml_dtypes
    bf = ml_dtypes.bfloat16
    f8 = ml_dtypes.float8_e4m3

    tr = np.asarray(inputs["token_reps"], dtype=np.float32)
    span_ids = np.asarray(inputs["span_ids"]).astype(np.int64)
    pe = np.asarray(inputs["pe"], dtype=np.float32)
    q0 = np.asarray(inputs["dummy_query"], dtype=np.float32)
    in_w = np.asarray(inputs["in_proj_w"], dtype=np.float32)
    in_b = np.asarray(inputs["in_proj_b"], dtype=np.float32)
    wo = np.asarray(inputs["out_proj_w"], dtype=np.float32)
    bo = np.asarray(inputs["out_proj_b"], dtype=np.float32)
    g = np.asarray(inputs["norm_g"], dtype=np.float32)
    bb = np.asarray(inputs["norm_b"], dtype=np.float32)
    w1 = np.asarray(inputs["ffn_w1"], dtype=np.float32)
    b1 = np.asarray(inputs["ffn_b1"], dtype=np.float32)
    w2 = np.asarray(inputs["ffn_w2"], dtype=np.float32)
    b2 = np.asarray(inputs["ffn_b2"], dtype=np.float32)

    Wq, Wk, Wv = in_w[0:H], in_w[H:2 * H], in_w[2 * H:3 * H]
    bq, bk, bv = in_b[0:H], in_b[H:2 * H], in_b[2 * H:3 * H]

    q = q0 @ Wq.T + bq
    qs = (q / np.sqrt(HD)).astype(np.float32)
    wq2 = np.stack([qs[n * HD:(n + 1) * HD] @ Wk[n * HD:(n + 1) * HD]
                    for n in range(NH)])                      # (12, 768)
    constv = np.array([qs[n * HD:(n + 1) * HD] @ bk[n * HD:(n + 1) * HD]
                       for n in range(NH)], dtype=np.float32)

    wvl = np.concatenate([Wv.T, wq2.T], axis=1).astype(np.float32)   # (768, 780)
    bvl = np.concatenate([np.zeros(H, np.float32), constv])[None, :]

    wout_c = wo - wo.mean(axis=0, keepdims=True)
    wout_ct = np.ascontiguousarray(wout_c.T).astype(np.float32)      # (768, 768)
    # fold V-bias through the (centered) out-projection into the LN1 bias
    b1c_full = bo + q0 + bv @ wout_c.T
    b1c = (b1c_full - b1c_full.mean()).astype(np.float32)

    ssel = np.zeros((NH, H), dtype=np.float32)
    for n in range(NH):
        ssel[n, n * HD:(n + 1) * HD] = 1.0

    # packed FFN weights:
    w2ct = w2.T - w2.T.mean(axis=1, keepdims=True)     # (INTERMED, H)
    if FP8:
        w1T = (w1.T * S1).reshape(HC // 2, 2, 128, NG, WG * 128)
        w1tp = np.ascontiguousarray(
            w1T.transpose(3, 2, 0, 1, 4)).astype(f8)   # (g, p, c2, two, 512)
        b2c = ((b2 - b2.mean()) * S2).astype(np.float32)
        w2T = (w2ct * S2).reshape(NG, WG // 2, 2, 128, H)
        w2tp = np.ascontiguousarray(
            w2T.transpose(0, 3, 1, 2, 4)).astype(f8)   # (g, p, l2, two, h)
    else:
        w1T = w1.T.reshape(HC, 128, NG, WG * 128)      # (c, p, g, l*128+n)
        w1tp = np.ascontiguousarray(
            w1T.transpose(2, 1, 0, 3)).astype(bf)      # (g, p, c, 512)
        b2c = (b2 - b2.mean()).astype(np.float32)
        w2T = w2ct.reshape(NG, WG, 128, H)             # (g, l, p, h)
        w2tp = np.ascontiguousarray(
            w2T.transpose(0, 2, 1, 3)).astype(bf)      # (g, p, l, h)

    x = tr + pe[None, :T]                              # (B, T, H)
    xTs = [np.ascontiguousarray(x[b].T).astype(bf) for b in range(B)]

    starts_all = span_ids[..., 0].astype(np.float32)   # (B, S)
    lens_all = (span_ids[..., 1] - span_ids[..., 0]).astype(np.float32)
    ends_all = starts_all + lens_all

    # per-core token windows (spans already sorted by start)
    sorted_idx, t0s, wmax = [], [], 1
    for core in range(NCORES):
        b = core // (NCORES // B)
        idx = per_core_idx[core]
        sorted_idx.append(idx)
        if len(idx):
            t0 = int(starts_all[b, idx].min())
            t1 = int(ends_all[b, idx].max())
        else:
            t0, t1 = 0, 1
        t0s.append(t0)
        wmax = max(wmax, t1 - t0)
    TCL = min((wmax + 127) // 128, (T + 127) // 128)
    TW = TCL * 128

    shared = dict(wvl=wvl.astype(bf), bvl=bvl.astype(bf),
                  ssel=ssel, wout=wout_ct.astype(bf),
                  b1c=b1c, gco=g, bco=bb, w1t=w1tp, b1r=b1, w2t=w2tp, b2c=b2c,
                  onesv=np.ones(128, dtype=np.float32),
                  onesb=np.ones(128, dtype=bf))
    in_maps = []
    for core in range(NCORES):
        b = core // (NCORES // B)
        idx = sorted_idx[core]
        t0 = t0s[core]
        st = np.full(C, float(t0), np.float32)
        en = np.full(C, float(t0 + 1), np.float32)  # pad spans: 1 token
        st[:len(idx)] = starts_all[b, idx]
        en[:len(idx)] = ends_all[b, idx]
        xw = np.zeros((H, TW), dtype=bf)
        hi = min(T, t0 + TW)
        xw[:, :hi - t0] = xTs[b][:, t0:hi]
        iota = (float(t0) + np.arange(128, dtype=np.float32)[:, None]
                + 128.0 * np.arange(TCL, dtype=np.float32)[None, :])
        m = dict(shared)
        m["xT"] = xw
        m["iota"] = np.ascontiguousarray(iota)
        m["starts"] = np.ascontiguousarray(st[None, :])
        m["ends"] = np.ascontiguousarray(en[None, :])
        in_maps.append(m)
    return in_maps, TCL, sorted_idx


def kernel(**inputs) -> np.ndarray:
    global _COMPILED
    from concourse.bass_utils import run_bass_kernel_spmd

    per_core_idx, C, scatter = _plan(inputs["span_masks"], inputs["span_ids"])
    gbi = (np.allclose(np.asarray(inputs["norm_g"], dtype=np.float32), 1.0)
           and np.allclose(np.asarray(inputs["norm_b"], dtype=np.float32), 0.0))
    in_maps, TCL, sorted_idx = _host_prepare(inputs, per_core_idx, C)
    key = (C, TCL, gbi, FP8)
    if key not in _COMPILED:
        _COMPILED[key] = _build(C, TCL, gb_identity=gbi, fp8=FP8)
    nc = _COMPILED[key]
    res = run_bass_kernel_spmd(nc, in_maps, core_ids=list(range(NCORES)))
    full = np.zeros((B, S, H), dtype=np.float32)
    cpb = NCORES // B
    for b in range(B):
        # concat the batch's unique-span outputs in (sorted) unique order,
        # then fan out to all duplicate original spans
        outs = []
        for k in range(cpb):
            core = b * cpb + k
            n = len(sorted_idx[core])
            o = np.asarray(res.results[core]["out"]).astype(np.float32)
            outs.append(o[:, :n].T)                    # [n, H]
        uniq_out = np.concatenate(outs, axis=0)
        idx, inv = scatter[b]
        full[b, idx] = uniq_out[inv]
    return full


# revision 61
# speedup vs baseline: 1.0572x; 1.0572x over previous
"""Trainium2 Bass kernel for AttentionPooling (ragged span attention pooling).

Math restructuring (vs the reference's gather-then-project):
  - K/V projections are computed once per unique token, not per gathered span
    token.
  - The query is a single shared vector, so per-span softmax factorizes:
        attn[s,n,l] = e[start_s+l, n] / Z[s,n],   e[t,n] = exp(q_n . k_{t,n} / 8)
        Z[s,n]      = sum_{t in span_s} e[t,n]
    Hence  ctx[s] = (1/Z[s]) * sum_t W[t,s] * (e[t] (x) V[t])  with the SAME 0/1
    banded window matrix W for all heads -> one dense matmul per core.
  - Everything runs feature-major (feature dim on partitions, spans on the free
    dim) end-to-end; the final output is written feature-major [H, C] and the
    host transposes (free).
  - Masked-out spans are dropped on the host (span compaction) AND spans are
    sorted by start so each core covers a narrow token window: V'/ctx/window
    work scales with the window (TCL*128 tokens), not the full T=512.
  - V'-bias (bv) is folded through the out-projection into the LN1 bias.
  - LayerNorms use centered-weight tricks (wout/W2 columns centered on the
    host) so no on-device mean subtraction is needed (gamma=1/beta=0 path).
  - Attention path in fp32r/bf16; FFN in fp8e4 (DoubleRow, 2 K-chunks per
    matmul) with fp32 PSUM accumulation, weights pre-scaled by 16/32 on the
    host (scales folded into the relu and the LN2 scale-invariance).

Sharding: unmasked spans of batch 0 (sorted by start) split over cores 0-3,
batch 1 over cores 4-7. Weights replicated.
"""

import sys
import numpy as np

if "/opt/trn_rl_repo" not in sys.path:
    sys.path.insert(0, "/opt/trn_rl_repo")

B, T, S, H, L, NH = 2, 512, 2048, 768, 32, 12
HD = H // NH            # 64
INTERMED = 4 * H        # 3072
NCORES = 8
HC = H // 128            # 6 feature chunks
IC = INTERMED // 128     # 24 intermediate chunks
VW = H + NH              # 780: [e-scaled V | e]
WG = 4                   # i-chunks per streamed weight group
NG = IC // WG            # 6 weight groups
NP = IC // 2             # 12 i-chunk pairs (fp8 DoubleRow)
EPS = 1e-5

FP8 = False              # fp8e4 DoubleRow FFN (measured: rel err 5e-2 AND
                         # DoubleRow MMs run at 2N cycles here -> no win)
S1 = 16.0                # host scale on W1 (fp8 range); undone in the relu
S2 = 32.0                # host scale on W2; folded via LN2 scale-invariance

_COMPILED = {}


def _build(C, TCL, gb_identity=False, fp8=FP8):
    """C = spans per core, TCL = 128-token chunks per core (compile-time)."""
    import concourse.bacc as bacc
    import concourse.tile as tile
    from concourse import mybir
    from concourse.alu_op_type import AluOpType as Op

    f32 = mybir.dt.float32
    f32r = mybir.dt.float32r
    bf16 = mybir.dt.bfloat16
    fp8e4 = mybir.dt.float8e4
    Act = mybir.ActivationFunctionType
    DR = mybir.MatmulPerfMode.DoubleRow
    mm_dt = fp8e4 if fp8 else bf16

    nc = bacc.Bacc("TRN2", target_bir_lowering=False, debug=False, num_devices=NCORES)

    def din(name, shape, dt=f32):
        return nc.dram_tensor(name, list(shape), dt, kind="ExternalInput").ap()

    TW = TCL * 128                    # token window size
    xT = din("xT", [H, TW], bf16)     # (x = token_reps + pe), transposed window
    wvl = din("wvl", [H, VW], bf16)   # [Wv.T | wq2.T]
    bvl = din("bvl", [1, VW], bf16)   # [0 | q.bk per head] (bv folded into b1c)
    starts = din("starts", [1, C])    # span starts (f32, global token ids)
    ends = din("ends", [1, C])        # span start + len (f32)
    iot = din("iota", [128, TCL])     # t_global per (partition, t-chunk)
    ssel = din("ssel", [NH, H])       # head selector: ssel[n,h'] = (h'//64==n)
    wout = din("wout", [H, H], bf16)  # (Wout - colmean(Wout)).T  [h', h]
    b1c = din("b1c", [H])             # centered (out_b + bv@WoutC.T + query)
    gco = din("gco", [H])             # norm gamma
    bco = din("bco", [H])             # norm beta
    if fp8:
        w1t = din("w1t", [NG, 128, HC // 2, 2, WG * 128], fp8e4)
        w2t = din("w2t", [NG, 128, WG // 2, 2, H], fp8e4)
    else:
        w1t = din("w1t", [NG, 128, HC, WG * 128], bf16)
        w2t = din("w2t", [NG, 128, WG, H], bf16)
    b1r = din("b1r", [INTERMED])      # ffn_b1
    b2c = din("b2c", [H])             # centered ffn_b2, pre-scaled by S2
    onesv = din("onesv", [128])       # ones (f32r matmul operand source)
    onesb = din("onesb", [128], bf16)  # ones (bf16)


    out = nc.dram_tensor("out", [H, C], bf16, kind="ExternalOutput").ap()

    with tile.TileContext(nc) as tc:
        with (
            tc.tile_pool(name="consts", bufs=1) as cp,
            tc.tile_pool(name="x1keep", bufs=1) as x1p,
            tc.tile_pool(name="w1s", bufs=3) as w1p,
            tc.tile_pool(name="w2s", bufs=6) as w2p,
        ):
            # ---- small consts; attention-critical ones FIRST (the gpsimd
            # SWDGE queue serializes ~1-2us per transfer, so queue position
            # is arrival time) ----
            ones1 = cp.tile([1, 128], f32r)      # K=1 matmul lhsT
            nc.gpsimd.dma_start(ones1[:], onesv.unsqueeze(0).bitcast(f32r))
            starts_r = cp.tile([1, C], f32r)
            nc.gpsimd.dma_start(starts_r[:], starts.bitcast(f32r))
            ends_r = cp.tile([1, C], f32r)
            nc.gpsimd.dma_start(ends_r[:], ends.bitcast(f32r))
            iota_sb = cp.tile([128, TCL], f32)
            nc.gpsimd.dma_start(iota_sb[:], iot)
            ssel_sb = cp.tile([NH, H], f32r)
            nc.gpsimd.dma_start(ssel_sb[:], ssel.bitcast(f32r))
            ones1w = cp.tile([1, 512], bf16)     # warmup rhs
            nc.vector.memset(ones1w, 1.0)
            ones1b = cp.tile([1, 128], bf16)
            nc.gpsimd.dma_start(ones1b[:], onesb.unsqueeze(0))
            bvl_sb = cp.tile([1, VW], bf16)
            nc.gpsimd.dma_start(bvl_sb[:], bvl)
            eps1 = cp.tile([1, 1], f32)
            nc.vector.memset(eps1, EPS)
            onescol = cp.tile([128, 1], f32r)    # partition-colsum lhsT
            nc.gpsimd.dma_start(onescol[:], onesv.unsqueeze(1).bitcast(f32r))
            gcol = cp.tile([128, HC], f32)      # gamma as per-partition cols
            nc.gpsimd.dma_start(gcol[:], gco.rearrange("(c p) -> p c", p=128))
            bcol = cp.tile([128, HC], f32)
            nc.gpsimd.dma_start(bcol[:], bco.rearrange("(c p) -> p c", p=128))
            b1ccol = cp.tile([128, HC], f32)
            nc.gpsimd.dma_start(b1ccol[:], b1c.rearrange("(c p) -> p c", p=128))
            b1col = cp.tile([128, IC], f32)
            nc.gpsimd.dma_start(b1col[:], b1r.rearrange("(c p) -> p c", p=128))
            b2ccol = cp.tile([128, HC], f32)
            nc.gpsimd.dma_start(b2ccol[:], b2c.rearrange("(c p) -> p c", p=128))
            eps2s = cp.tile([1, 1], f32)    # LN2 eps (scaled in fp8 mode)
            nc.vector.memset(eps2s, EPS * (S2 * S2 if fp8 else 1.0))

            # x1 kept (up to) three ways: matmul operand, unit-scale residual
            # basis, S2-scaled residual for the (scaled-W2) FFN2 add.
            x1b = x1p.tile([128, HC, C], bf16, name="x1b")
            if fp8:
                x1mm = x1p.tile([128, HC, C], mm_dt, name="x1mm")
                x1s2 = x1p.tile([128, HC, C], bf16, name="x1s2")
            else:
                x1mm, x1s2 = x1b, x1b

            # FFN weight tiles allocated early (stable addresses); all of W2
            # goes on the gpsimd queue (idle during attention), W1 groups 0-2
            # trail the attention-critical loads on sync.
            if fp8:
                w1g = [w1p.tile([128, HC // 2, 2, WG * 128], fp8e4, tag="w1",
                                name=f"w1g{g}") for g in range(3)]
                w2g = [w2p.tile([128, WG // 2, 2, H], fp8e4, tag="w2",
                                name=f"w2g{g}") for g in range(NG)]
            else:
                w1g = [w1p.tile([128, HC, WG * 128], bf16, tag="w1",
                                name=f"w1g{g}") for g in range(3)]
                w2g = [w2p.tile([128, WG, H], bf16, tag="w2",
                                name=f"w2g{g}") for g in range(NG)]

            # ---------------- attention (feature-major) ----------------
            with (
                tc.tile_pool(name="attn", bufs=1) as ap_,
                tc.tile_pool(name="attn_s", bufs=2) as asml,
            ):
                xTc = [ap_.tile([128, TW], bf16, name=f"xTc{c}")
                       for c in range(HC)]
                wvlc = [ap_.tile([128, VW], bf16, name=f"wvlc{c}")
                        for c in range(HC)]
                xTr = xT.rearrange("(c p) t -> c p t", p=128)
                wvlr = wvl.rearrange("(c p) n -> c p n", p=128)
                # attention-critical loads first, round-robin on two queues
                qs = [nc.sync, nc.scalar]
                for c in range(HC):
                    qs[0].dma_start(wvlc[c][:], wvlr[c])
                    qs[1].dma_start(xTc[c][:], xTr[c])

                # separate tiles for e*V and e: keeps the Z matmul (reads e)
                # from false-serializing behind the e*V DVE multiply (Tile
                # tracks deps per tile)
                veV = ap_.tile([128, TCL, H], bf16)   # e*V, token-major
                veE = ap_.tile([128, TCL, NH], bf16)  # e,   token-major
                wt = ap_.tile([128, TCL, C], bf16)    # W[t, s] 0/1 window
                ctxN = ap_.tile([128, HC, C], bf16)   # normalized ctx
                ycs = ap_.tile([128, HC, C], bf16)    # out_proj + b1c

                with (
                    tc.tile_pool(name="psAux", bufs=1, space="PSUM") as psAux,
                    tc.tile_pool(name="psV", bufs=2, space="PSUM") as psV,
                ):
                    # warm up the PE (HAM clock gate) while the loads land;
                    # also preload the Exp activation table.
                    wf = asml.tile([1, 128], bf16, tag="wf")
                    nc.vector.memset(wf, 1.0)
                    dexp = asml.tile([1, 1], f32, tag="dexp")
                    nc.scalar.activation(dexp[:], eps1[:], Act.Exp)
                    dum = psAux.tile([128, 512], f32, tag="warm")
                    for k in range(8):
                        nc.tensor.matmul(dum[:], wf[:], ones1w[:],
                                         start=(k == 0), stop=(k == 7))
                    # broadcast span starts/ends across partitions on the PE
                    startsB = psAux.tile([128, C], f32, tag="sb",
                                         name="startsB")
                    nc.tensor.matmul(startsB[:], ones1[:], starts_r[:],
                                     start=True, stop=True)
                    endsB = psAux.tile([128, C], f32, tag="eb",
                                       name="endsB")
                    nc.tensor.matmul(endsB[:], ones1[:], ends_r[:],
                                     start=True, stop=True)
                    # remaining weights on the two HWDGE queues (keeping the
                    # gpsimd queue empty so PSUM pool releases aren't stuck
                    # behind slow SWDGE DMA waits)
                    wout_sb = ap_.tile([128, HC, H], bf16)
                    nc.sync.dma_start(wout_sb[:],
                                      wout.rearrange("(c p) n -> p c n", p=128))
                    for g in range(3):
                        nc.sync.dma_start(w1g[g][:], w1t[g])
                    for g in range(NG):
                        qs[g % 2].dma_start(w2g[g][:], w2t[g])

                    # W[t, s] = (start_s <= t) & (t < end_s)  (needs no x)
                    for t in range(TCL):
                        lt = asml.tile([128, C], f32, tag="uexp", name="lt")
                        nc.vector.tensor_scalar(
                            lt[:], endsB[:], iota_sb[:, t:t + 1], None,
                            Op.is_gt)
                        nc.vector.scalar_tensor_tensor(
                            wt[:, t, :], startsB[:], iota_sb[:, t:t + 1],
                            lt[:], Op.is_le, Op.mult)

                    # V' = x @ [Wv.T | wq2.T] (+ e-logit bias)  (token-major)
                    for t in range(TCL):
                        vp = psV.tile([128, VW], f32, tag="vp")
                        for lo, hi in ((0, 512), (512, VW)):
                            for c in range(HC):
                                nc.tensor.matmul(
                                    vp[:, lo:hi],
                                    xTc[c][:, t * 128:(t + 1) * 128],
                                    wvlc[c][:, lo:hi],
                                    start=(c == 0),
                                    stop=(c == HC - 1 and hi != VW),
                                )
                        nc.tensor.matmul(
                            vp[:, H:VW], ones1b[:], bvl_sb[:, H:VW],
                            start=False, stop=True,
                        )
                        # e = exp(logits)
                        nc.scalar.activation(veE[:, t, :], vp[:, H:VW],
                                             Act.Exp)
                        # veV[:, t] = V * e (per-head broadcast of e)
                        e_b = veE[:, t, :].unsqueeze(2).broadcast_to(
                            [128, NH, HD])
                        nc.vector.tensor_tensor(
                            veV[:, t, :].rearrange("p (n d) -> p n d", d=HD),
                            vp[:, 0:H].rearrange("p (n d) -> p n d", d=HD),
                            e_b, Op.mult,
                        )
                    # preload the Sqrt table now that Exp is done with ACT
                    # (read a ve value so this schedules AFTER the Exp ops)
                    dsq = asml.tile([1, 1], f32, tag="dexp", name="dsq")
                    nc.scalar.activation(dsq[:], veE[0:1, TCL - 1, 0:1],
                                         Act.Sqrt)

                with (
                    tc.tile_pool(name="psMM", bufs=6, space="PSUM") as psMM,
                    tc.tile_pool(name="psS", bufs=2, space="PSUM") as psS,
                ):
                    cps = [psMM.tile([128, C], f32, tag="mm512",
                                     name=f"cp{c}") for c in range(HC)]
                    # ctx t=0 first: the PE stays busy in-order while Z's
                    # bank may still be held by the (late-released) V' buffer
                    for c in range(HC):
                        nc.tensor.matmul(
                            cps[c][:], veV[:, 0, c * 128:(c + 1) * 128],
                            wt[:, 0, :], start=True, stop=(TCL == 1))
                    # Z[n, s] then u = 1/Z  (Z >= exp(tiny logit) ~ 1 always:
                    # every span keeps >= 1 in-window token; approx_fast is
                    # ~18 bits, ample for an attn scale kept in bf16 later)
                    zp = psS.tile([NH, C], f32, tag="small", name="zp")
                    for t in range(TCL):
                        nc.tensor.matmul(
                            zp[:], veE[:, t, :], wt[:, t, :],
                            start=(t == 0), stop=(t == TCL - 1))
                    uf = asml.tile([NH, C], f32, tag="uf", name="uf")
                    nc.vector.reciprocal_approx_fast(uf[:], zp[:])
                    u_sb = asml.tile([NH, C], f32r, tag="u")
                    with nc.allow_low_precision(reason="f32r is fp32-width"):
                        nc.vector.tensor_copy(u_sb[:], uf[:])

                    # remaining ctx accumulation + u broadcast/apply
                    uexps = []
                    for c in range(HC):
                        cp_ = cps[c]
                        for t in range(1, TCL):
                            nc.tensor.matmul(
                                cp_[:], veV[:, t, c * 128:(c + 1) * 128],
                                wt[:, t, :],
                                start=False, stop=(t == TCL - 1))
                        up = psS.tile([128, C], f32, tag="small",
                                      name=f"up{c}")
                        nc.tensor.matmul(
                            up[:], ssel_sb[:, c * 128:(c + 1) * 128],
                            u_sb[:], start=True, stop=True)
                        uexp = asml.tile([128, C], bf16, tag="uexp",
                                         name=f"uexp{c}")
                        nc.scalar.activation(uexp[:], up[:], Act.Identity)
                        uexps.append(uexp)
                        if c >= 1:
                            nc.vector.tensor_tensor(ctxN[:, c - 1, :],
                                                    cps[c - 1][:],
                                                    uexps[c - 1][:], Op.mult)
                    nc.vector.tensor_tensor(ctxN[:, HC - 1, :], cps[HC - 1][:],
                                            uexps[HC - 1][:], Op.mult)

                    # out_proj (centered weights) + LN1, feature-major
                    varp = psS.tile([1, C], f32, tag="small", name="varp")
                    sqs = []
                    for m in range(HC):
                        aop = psMM.tile([128, C], f32, tag="mm512",
                                        name=f"aop{m}")
                        for c in range(HC):
                            nc.tensor.matmul(
                                aop[:], wout_sb[:, c, m * 128:(m + 1) * 128],
                                ctxN[:, c, :],
                                start=(c == 0), stop=(c == HC - 1))
                        # yc = aop + centered bias (per-partition) on ACT
                        nc.scalar.activation(ycs[:, m, :], aop[:],
                                             Act.Identity,
                                             bias=b1ccol[:, m:m + 1])
                        sq = asml.tile([128, C], f32r, tag="sq",
                                       name=f"sq{m}")
                        with nc.allow_low_precision(reason="f32r fp32-width"):
                            nc.vector.tensor_tensor(sq[:], ycs[:, m, :],
                                                    ycs[:, m, :], Op.mult)
                        sqs.append(sq)
                        nc.tensor.matmul(
                            varp[:], onescol[:], sq[:],
                            start=(m == 0), stop=(m == HC - 1))
                    # keep the PE warm through the serial rstd chain (a >3.4us
                    # idle gap would re-throttle the PE clock to half rate);
                    # reading ycs[5] anchors these INTO the chain window --
                    # dependency-free matmuls would get scheduled early
                    wrm = psS.tile([128, C], f32, tag="small", name="wrm")
                    for k in range(10):
                        nc.tensor.matmul(wrm[:], ycs[:, HC - 1, 0:128],
                                         ycs[:, HC - 1, :],
                                         start=(k == 0), stop=(k == 9))
                    # rstd = 1/sqrt(varp/H + eps); approx_fast (~18 bits) is
                    # ample for a scale kept in bf16 downstream
                    sd = asml.tile([1, C], f32, tag="sd")
                    nc.scalar.activation(sd[:], varp[:], Act.Sqrt,
                                         bias=eps1[:], scale=1.0 / H)
                    rf = asml.tile([1, C], f32, tag="rf2", name="rf")
                    nc.vector.reciprocal_approx_fast(rf[:], sd[:])
                    rstd = asml.tile([1, C], bf16, tag="rstd")
                    with nc.allow_low_precision(reason="per-span scale; "
                                                "bf16 downstream"):
                        nc.vector.tensor_copy(rstd[:], rf[:])
                    rp = psS.tile([128, C], f32, tag="small", name="rp")
                    nc.tensor.matmul(rp[:], ones1b[:], rstd[:],
                                     start=True, stop=True)
                    # second warm group: bridges rp -> x1b[0] -> first FFN1 MM
                    # (anchored on rstd so it lands exactly in that window)
                    wrm_b = psS.tile([128, C], f32, tag="small", name="wrm_b")
                    for k in range(4):
                        nc.tensor.matmul(wrm_b[:], ones1b[:], rstd[:],
                                         start=(k == 0), stop=(k == 3))
                    rpb = asml.tile([128, C], bf16, tag="rpb")
                    nc.scalar.activation(rpb[:], rp[:], Act.Identity)
                    for m in range(HC):
                        if gb_identity:
                            # chunk 0 straight off the PSUM broadcast (rpb
                            # lags by one ACT op); rest via bf16 2x-mode
                            if m == 0:
                                nc.vector.tensor_tensor(x1b[:, m, :],
                                                        ycs[:, m, :],
                                                        rp[:], Op.mult)
                            else:
                                nc.vector.tensor_tensor(x1b[:, m, :],
                                                        ycs[:, m, :],
                                                        rpb[:], Op.mult)
                        else:
                            tmp = asml.tile([128, C], f32, tag="tmp",
                                            name="tmp")
                            nc.vector.tensor_tensor(tmp[:], ycs[:, m, :],
                                                    rp[:], Op.mult)
                            nc.scalar.activation(x1b[:, m, :], tmp[:],
                                                 Act.Identity,
                                                 scale=gcol[:, m:m + 1],
                                                 bias=bcol[:, m:m + 1])
                        if fp8:
                            # fp8 matmul operand (ACT) + S2-scaled residual
                            nc.scalar.activation(x1mm[:, m, :], x1b[:, m, :],
                                                 Act.Copy)
                            nc.vector.tensor_scalar(x1s2[:, m, :],
                                                    x1b[:, m, :], S2, None,
                                                    Op.mult)

            # ---------------- FFN + LN2 (feature-major, pipelined) --------
            with (
                tc.tile_pool(name="ffn", bufs=1) as fp_,
                tc.tile_pool(name="ffn_s", bufs=2) as fsml,
                tc.tile_pool(name="ffn_o", bufs=6) as fout,
                tc.tile_pool(name="psH", bufs=2, space="PSUM") as psH,
                tc.tile_pool(name="psF", bufs=1, space="PSUM") as psF,
            ):
                h1s = fp_.tile([128, IC, C], mm_dt)
                zF = fp_.tile([128, HC, C], bf16)
                h2T = [psF.tile([128, C], f32, tag=f"h2_{m}", name=f"h2T{m}")
                       for m in range(HC)]

                x1op = x1mm if fp8 else x1b

                def ffn2_pair(ip):
                    g, lp = divmod(ip, WG // 2)
                    for m in range(HC):
                        nc.tensor.matmul(
                            h2T[m][:],
                            w2g[g][:, lp, :, m * 128:(m + 1) * 128],
                            h1s[:, 2 * ip:2 * ip + 2, :],
                            start=(ip == 0), stop=False, perf_mode=DR)

                def ffn2_single(i):
                    g, l = divmod(i, WG)
                    for m in range(HC):
                        nc.tensor.matmul(
                            h2T[m][:],
                            w2g[g][:, l, m * 128:(m + 1) * 128],
                            h1s[:, i, :],
                            start=(i == 0), stop=False)

                NTAIL = 7        # bf16: last i-chunks run m-outer
                NTAILP = 4       # fp8: last i-chunk pairs run m-outer

                for i in range(IC):
                    g, l = divmod(i, WG)
                    if l == 0 and g >= 3:  # stream remaining W1 groups
                        if fp8:
                            w1g[g % 3] = w1p.tile(
                                [128, HC // 2, 2, WG * 128], fp8e4,
                                tag="w1", name=f"w1g{g}")
                        else:
                            w1g[g % 3] = w1p.tile(
                                [128, HC, WG * 128], bf16,
                                tag="w1", name=f"w1g{g}")
                        nc.sync.dma_start(w1g[g % 3][:], w1t[g])
                    w1 = w1g[g % 3]
                    h1p = psH.tile([128, C], f32, tag="h1p")
                    if fp8:
                        for cp2 in range(HC // 2):
                            nc.tensor.matmul(
                                h1p[:],
                                w1[:, cp2, :, l * 128:(l + 1) * 128],
                                x1op[:, 2 * cp2:2 * cp2 + 2, :],
                                start=(cp2 == 0), stop=(cp2 == HC // 2 - 1),
                                perf_mode=DR)
                    else:
                        for c in range(HC):
                            nc.tensor.matmul(
                                h1p[:], w1[:, c, l * 128:(l + 1) * 128],
                                x1op[:, c, :],
                                start=(c == 0), stop=(c == HC - 1))
                    nc.scalar.activation(h1s[:, i, :], h1p[:], Act.Relu,
                                         bias=b1col[:, i:i + 1],
                                         scale=1.0 / S1 if fp8 else 1.0)
                    if fp8:
                        if i % 2 == 1 and 1 <= i // 2 <= NP - NTAILP:
                            ffn2_pair(i // 2 - 1)
                    else:
                        if 1 <= i <= IC - NTAIL:
                            ffn2_single(i - 1)

                # LN2 (no mean: W2/b2 centered, x1 zero-mean) + output.
                # Last chunks run m-outer: h2T[m] finishes early so its
                # z/square/var chain overlaps the remaining matmuls.
                varp2 = psH.tile([1, C], f32, tag="h1p", name="varp2")
                sq2s = []
                for m in range(HC):
                    if fp8:
                        for ip in range(NP - NTAILP, NP):
                            g, lp = divmod(ip, WG // 2)
                            nc.tensor.matmul(
                                h2T[m][:],
                                w2g[g][:, lp, :, m * 128:(m + 1) * 128],
                                h1s[:, 2 * ip:2 * ip + 2, :],
                                start=False, stop=(ip == NP - 1),
                                perf_mode=DR)
                    else:
                        for i in range(IC - NTAIL, IC):
                            g, l = divmod(i, WG)
                            nc.tensor.matmul(
                                h2T[m][:],
                                w2g[g][:, l, m * 128:(m + 1) * 128],
                                h1s[:, i, :],
                                start=False, stop=(i == IC - 1))
                    # z = (h2 + S2*b2c) + S2*x1  (one DVE op, feature-major)
                    nc.vector.scalar_tensor_tensor(
                        zF[:, m, :], h2T[m][:], b2ccol[:, m:m + 1],
                        x1s2[:, m, :], Op.add, Op.add)
                    sq2 = fsml.tile([128, C], f32r, tag="sq2", name=f"sq2{m}")
                    with nc.allow_low_precision(reason="f32r is fp32-width"):
                        nc.scalar.activation(sq2[:], zF[:, m, :], Act.Square)
                    sq2s.append(sq2)
                    if m >= 2:
                        nc.tensor.matmul(
                            varp2[:], onescol[:], sq2s[m - 2][:],
                            start=(m - 2 == 0), stop=False)
                for m in (HC - 2, HC - 1):
                    nc.tensor.matmul(
                        varp2[:], onescol[:], sq2s[m][:],
                        start=False, stop=(m == HC - 1))
                wrm2 = psH.tile([128, C], f32, tag="h1p", name="wrm2")
                for k in range(10):
                    nc.tensor.matmul(wrm2[:], zF[:, HC - 1, 0:128],
                                     zF[:, HC - 1, :],
                                     start=(k == 0), stop=(k == 9))
                sd2 = fsml.tile([1, C], f32, tag="sd2")
                nc.scalar.activation(sd2[:], varp2[:], Act.Sqrt,
                                     bias=eps2s[:], scale=1.0 / H)
                rf2 = fsml.tile([1, C], f32, tag="rf2b", name="rf2")
                nc.vector.reciprocal_approx_fast(rf2[:], sd2[:])
                rstd2 = fsml.tile([1, C], bf16, tag="rstd2")
                with nc.allow_low_precision(reason="output is bf16 anyway"):
                    nc.vector.tensor_copy(rstd2[:], rf2[:])
                rp2 = psH.tile([128, C], f32, tag="h1p", name="rp2")
                nc.tensor.matmul(rp2[:], ones1b[:], rstd2[:],
                                 start=True, stop=True)
                rp2b = fsml.tile([128, C], bf16, tag="rp2b")
                nc.scalar.activation(rp2b[:], rp2[:], Act.Identity)
                oqs = [nc.sync, nc.scalar, nc.gpsimd]
                for m in range(HC):
                    o = fout.tile([128, C], bf16, tag="o", name=f"o{m}")
                    if gb_identity:
                        # chunk 0 straight off the PSUM broadcast (rp2b lags
                        # by one ACT op); rest via bf16 2x-mode
                        nc.vector.tensor_tensor(o[:], zF[:, m, :],
                                                rp2[:] if m == 0 else rp2b[:],
                                                Op.mult)
                    else:
                        on = fout.tile([128, C], f32, tag="on", name=f"on{m}")
                        nc.vector.tensor_tensor(on[:], zF[:, m, :],
                                                rp2b[:], Op.mult)
                        nc.scalar.activation(o[:], on[:], Act.Identity,
                                             scale=gcol[:, m:m + 1],
                                             bias=bcol[:, m:m + 1])
                    oqs[m % 2].dma_start(out[m * 128:(m + 1) * 128, :], o[:])
    nc.compile()
    return nc


def _plan(span_masks, span_ids):
    """Compact unmasked spans, dedup identical (start,end) pairs, sort by
    start, split each batch's unique spans across its 4 cores (so each core
    covers a narrow token window). Returns (per_core_idx, C, scatter) where
    per_core_idx holds representative original span indices and scatter maps
    every unmasked original span to (its batch's unique-span position)."""
    masks = np.asarray(span_masks)
    sid = np.asarray(span_ids)
    per_core_idx, scatter = [], []
    cpb = NCORES // B
    for b in range(B):
        idx = np.nonzero(masks[b])[0]
        key = sid[b, idx, 0] * 1024 + sid[b, idx, 1]   # start-major sort key
        uk, first, inv = np.unique(key, return_index=True,
                                   return_inverse=True)
        rep = idx[first]                               # one original per uniq
        per_core_idx.extend(np.array_split(rep, cpb))
        scatter.append((idx, inv))
    cmax = max(len(ix) for ix in per_core_idx)
    C = (cmax + 7) // 8 * 8
    C = min(max(C, 128), 512)
    return per_core_idx, C, scatter


def _host_prepare(inputs, per_core_idx, C):
    """Host-side packing: tiny index/weight reshapes, no heavy math.
    Returns (in_maps, TCL, sorted per_core_idx)."""
    import ml_dtypes
    bf = ml_dtypes.bfloat16
    f8 = ml_dtypes.float8_e4m3

    tr = np.asarray(inputs["token_reps"], dtype=np.float32)
    span_ids = np.asarray(inputs["span_ids"]).astype(np.int64)
    pe = np.asarray(inputs["pe"], dtype=np.float32)
    q0 = np.asarray(inputs["dummy_query"], dtype=np.float32)
    in_w = np.asarray(inputs["in_proj_w"], dtype=np.float32)
    in_b = np.asarray(inputs["in_proj_b"], dtype=np.float32)
    wo = np.asarray(inputs["out_proj_w"], dtype=np.float32)
    bo = np.asarray(inputs["out_proj_b"], dtype=np.float32)
    g = np.asarray(inputs["norm_g"], dtype=np.float32)
    bb = np.asarray(inputs["norm_b"], dtype=np.float32)
    w1 = np.asarray(inputs["ffn_w1"], dtype=np.float32)
    b1 = np.asarray(inputs["ffn_b1"], dtype=np.float32)
    w2 = np.asarray(inputs["ffn_w2"], dtype=np.float32)
    b2 = np.asarray(inputs["ffn_b2"], dtype=np.float32)

    Wq, Wk, Wv = in_w[0:H], in_w[H:2 * H], in_w[2 * H:3 * H]
    bq, bk, bv = in_b[0:H], in_b[H:2 * H], in_b[2 * H:3 * H]

    q = q0 @ Wq.T + bq
    qs = (q / np.sqrt(HD)).astype(np.float32)
    wq2 = np.stack([qs[n * HD:(n + 1) * HD] @ Wk[n * HD:(n + 1) * HD]
                    for n in range(NH)])                      # (12, 768)
    constv = np.array([qs[n * HD:(n + 1) * HD] @ bk[n * HD:(n + 1) * HD]
                       for n in range(NH)], dtype=np.float32)

    wvl = np.concatenate([Wv.T, wq2.T], axis=1).astype(np.float32)   # (768, 780)
    bvl = np.concatenate([np.zeros(H, np.float32), constv])[None, :]

    wout_c = wo - wo.mean(axis=0, keepdims=True)
    wout_ct = np.ascontiguousarray(wout_c.T).astype(np.float32)      # (768, 768)
    # fold V-bias through the (centered) out-projection into the LN1 bias
    b1c_full = bo + q0 + bv @ wout_c.T
    b1c = (b1c_full - b1c_full.mean()).astype(np.float32)

    ssel = np.zeros((NH, H), dtype=np.float32)
    for n in range(NH):
        ssel[n, n * HD:(n + 1) * HD] = 1.0

    # packed FFN weights:
    w2ct = w2.T - w2.T.mean(axis=1, keepdims=True)     # (INTERMED, H)
    if FP8:
        w1T = (w1.T * S1).reshape(HC // 2, 2, 128, NG, WG * 128)
        w1tp = np.ascontiguousarray(
            w1T.transpose(3, 2, 0, 1, 4)).astype(f8)   # (g, p, c2, two, 512)
        b2c = ((b2 - b2.mean()) * S2).astype(np.float32)
        w2T = (w2ct * S2).reshape(NG, WG // 2, 2, 128, H)
        w2tp = np.ascontiguousarray(
            w2T.transpose(0, 3, 1, 2, 4)).astype(f8)   # (g, p, l2, two, h)
    else:
        w1T = w1.T.reshape(HC, 128, NG, WG * 128)      # (c, p, g, l*128+n)
        w1tp = np.ascontiguousarray(
            w1T.transpose(2, 1, 0, 3)).astype(bf)      # (g, p, c, 512)
        b2c = (b2 - b2.mean()).astype(np.float32)
        w2T = w2ct.reshape(NG, WG, 128, H)             # (g, l, p, h)
        w2tp = np.ascontiguousarray(
            w2T.transpose(0, 2, 1, 3)).astype(bf)      # (g, p, l, h)

    x = tr + pe[None, :T]                              # (B, T, H)
    xTs = [np.ascontiguousarray(x[b].T).astype(bf) for b in range(B)]

    starts_all = span_ids[..., 0].astype(np.float32)   # (B, S)
    lens_all = (span_ids[..., 1] - span_ids[..., 0]).astype(np.float32)
    ends_all = starts_all + lens_all

    # per-core token windows (spans already sorted by start)
    sorted_idx, t0s, wmax = [], [], 1
    for core in range(NCORES):
        b = core // (NCORES // B)
        idx = per_core_idx[core]
        sorted_idx.append(idx)
        if len(idx):
            t0 = int(starts_all[b, idx].min())
            t1 = int(ends_all[b, idx].max())
        else:
            t0, t1 = 0, 1
        t0s.append(t0)
        wmax = max(wmax, t1 - t0)
    TCL = min((wmax + 127) // 128, (T + 127) // 128)
    TW = TCL * 128

    shared = dict(wvl=wvl.astype(bf), bvl=bvl.astype(bf),
                  ssel=ssel, wout=wout_ct.astype(bf),
                  b1c=b1c, gco=g, bco=bb, w1t=w1tp, b1r=b1, w2t=w2tp, b2c=b2c,
                  onesv=np.ones(128, dtype=np.float32),
                  onesb=np.ones(128, dtype=bf))
    in_maps = []
    for core in range(NCORES):
        b = core // (NCORES // B)
        idx = sorted_idx[core]
        t0 = t0s[core]
        st = np.full(C, float(t0), np.float32)
        en = np.full(C, float(t0 + 1), np.float32)  # pad spans: 1 token
        st[:len(idx)] = starts_all[b, idx]
        en[:len(idx)] = ends_all[b, idx]
        xw = np.zeros((H, TW), dtype=bf)
        hi = min(T, t0 + TW)
        xw[:, :hi - t0] = xTs[b][:, t0:hi]
        iota = (float(t0) + np.arange(128, dtype=np.float32)[:, None]
                + 128.0 * np.arange(TCL, dtype=np.float32)[None, :])
        m = dict(shared)
        m["xT"] = xw
        m["iota"] = np.ascontiguousarray(iota)
        m["starts"] = np.ascontiguousarray(st[None, :])
        m["ends"] = np.ascontiguousarray(en[None, :])
        in_maps.append(m)
    return in_maps, TCL, sorted_idx


def kernel(**inputs) -> np.ndarray:
    global _COMPILED
    from concourse.bass_utils import run_bass_kernel_spmd

    per_core_idx, C, scatter = _plan(inputs["span_masks"], inputs["span_ids"])
    gbi = (np.allclose(np.asarray(inputs["norm_g"], dtype=np.float32), 1.0)
           and np.allclose(np.asarray(inputs["norm_b"], dtype=np.float32), 0.0))
    in_maps, TCL, sorted_idx = _host_prepare(inputs, per_core_idx, C)
    key = (C, TCL, gbi, FP8)
    if key not in _COMPILED:
        _COMPILED[key] = _build(C, TCL, gb_identity=gbi, fp8=FP8)
    nc = _COMPILED[key]
    res = run_bass_kernel_spmd(nc, in_maps, core_ids=list(range(NCORES)))
    full = np.zeros((B, S, H), dtype=np.float32)
    cpb = NCORES // B
    for b in range(B):
        # concat the batch's unique-span outputs in (sorted) unique order,
        # then fan out to all duplicate original spans
        outs = []
        for k in range(cpb):
            core = b * cpb + k
            n = len(sorted_idx[core])
            o = np.asarray(res.results[core]["out"]).astype(np.float32)
            outs.append(o[:, :n].T)                    # [n, H]
        uniq_out = np.concatenate(outs, axis=0)
        idx, inv = scatter[b]
        full[b, idx] = uniq_out[inv]
    return full


# revision 62
# speedup vs baseline: 1.0708x; 1.0128x over previous
"""Trainium2 Bass kernel for AttentionPooling (ragged span attention pooling).

Math restructuring (vs the reference's gather-then-project):
  - K/V projections are computed once per unique token, not per gathered span
    token.
  - The query is a single shared vector, so per-span softmax factorizes:
        attn[s,n,l] = e[start_s+l, n] / Z[s,n],   e[t,n] = exp(q_n . k_{t,n} / 8)
        Z[s,n]      = sum_{t in span_s} e[t,n]
    Hence  ctx[s] = (1/Z[s]) * sum_t W[t,s] * (e[t] (x) V[t])  with the SAME 0/1
    banded window matrix W for all heads -> one dense matmul per core.
  - Everything runs feature-major (feature dim on partitions, spans on the free
    dim) end-to-end; the final output is written feature-major [H, C] and the
    host transposes (free).
  - Masked-out spans are dropped on the host (span compaction) AND spans are
    sorted by start so each core covers a narrow token window: V'/ctx/window
    work scales with the window (TCL*128 tokens), not the full T=512.
  - V'-bias (bv) is folded through the out-projection into the LN1 bias.
  - LayerNorms use centered-weight tricks (wout/W2 columns centered on the
    host) so no on-device mean subtraction is needed (gamma=1/beta=0 path).
  - Attention path in fp32r/bf16; FFN in fp8e4 (DoubleRow, 2 K-chunks per
    matmul) with fp32 PSUM accumulation, weights pre-scaled by 16/32 on the
    host (scales folded into the relu and the LN2 scale-invariance).

Sharding: unmasked spans of batch 0 (sorted by start) split over cores 0-3,
batch 1 over cores 4-7. Weights replicated.
"""

import sys
import numpy as np

if "/opt/trn_rl_repo" not in sys.path:
    sys.path.insert(0, "/opt/trn_rl_repo")

B, T, S, H, L, NH = 2, 512, 2048, 768, 32, 12
HD = H // NH            # 64
INTERMED = 4 * H        # 3072
NCORES = 8
HC = H // 128            # 6 feature chunks
IC = INTERMED // 128     # 24 intermediate chunks
VW = H + NH              # 780: [e-scaled V | e]
WG = 4                   # i-chunks per streamed weight group
NG = IC // WG            # 6 weight groups
NP = IC // 2             # 12 i-chunk pairs (fp8 DoubleRow)
EPS = 1e-5

FP8 = False              # fp8e4 DoubleRow FFN (measured: rel err 5e-2 AND
                         # DoubleRow MMs run at 2N cycles here -> no win)
S1 = 16.0                # host scale on W1 (fp8 range); undone in the relu
S2 = 32.0                # host scale on W2; folded via LN2 scale-invariance

_COMPILED = {}


def _build(C, TCL, gb_identity=False, fp8=FP8):
    """C = spans per core, TCL = 128-token chunks per core (compile-time)."""
    import concourse.bacc as bacc
    import concourse.tile as tile
    from concourse import mybir
    from concourse.alu_op_type import AluOpType as Op

    f32 = mybir.dt.float32
    f32r = mybir.dt.float32r
    bf16 = mybir.dt.bfloat16
    fp8e4 = mybir.dt.float8e4
    Act = mybir.ActivationFunctionType
    DR = mybir.MatmulPerfMode.DoubleRow
    mm_dt = fp8e4 if fp8 else bf16

    nc = bacc.Bacc("TRN2", target_bir_lowering=False, debug=False, num_devices=NCORES)

    def din(name, shape, dt=f32):
        return nc.dram_tensor(name, list(shape), dt, kind="ExternalInput").ap()

    TW = TCL * 128                    # token window size
    xT = din("xT", [H, TW], bf16)     # (x = token_reps + pe), transposed window
    wvl = din("wvl", [H, VW], bf16)   # [Wv.T | wq2.T]
    bvl = din("bvl", [1, VW], bf16)   # [0 | q.bk per head] (bv folded into b1c)
    starts = din("starts", [1, C])    # span starts (f32, global token ids)
    ends = din("ends", [1, C])        # span start + len (f32)
    iot = din("iota", [128, TCL])     # t_global per (partition, t-chunk)
    ssel = din("ssel", [NH, H])       # head selector: ssel[n,h'] = (h'//64==n)
    wout = din("wout", [H, H], bf16)  # (Wout - colmean(Wout)).T  [h', h]
    b1c = din("b1c", [H])             # centered (out_b + bv@WoutC.T + query)
    gco = din("gco", [H])             # norm gamma
    bco = din("bco", [H])             # norm beta
    if fp8:
        w1t = din("w1t", [NG, 128, HC // 2, 2, WG * 128], fp8e4)
        w2t = din("w2t", [NG, 128, WG // 2, 2, H], fp8e4)
    else:
        w1t = din("w1t", [NG, 128, HC, WG * 128], bf16)
        w2t = din("w2t", [NG, 128, WG, H], bf16)
    b1r = din("b1r", [INTERMED])      # ffn_b1
    b2c = din("b2c", [H])             # centered ffn_b2, pre-scaled by S2
    onesv = din("onesv", [128])       # ones (f32r matmul operand source)
    onesb = din("onesb", [128], bf16)  # ones (bf16)


    out = nc.dram_tensor("out", [H, C], bf16, kind="ExternalOutput").ap()

    with tile.TileContext(nc) as tc:
        with (
            tc.tile_pool(name="consts", bufs=1) as cp,
            tc.tile_pool(name="x1keep", bufs=1) as x1p,
            tc.tile_pool(name="w1s", bufs=3) as w1p,
            tc.tile_pool(name="w2s", bufs=6) as w2p,
        ):
            # ---- small consts; attention-critical ones FIRST (the gpsimd
            # SWDGE queue serializes ~1-2us per transfer, so queue position
            # is arrival time) ----
            ones1 = cp.tile([1, 128], f32r)      # K=1 matmul lhsT
            nc.gpsimd.dma_start(ones1[:], onesv.unsqueeze(0).bitcast(f32r))
            starts_r = cp.tile([1, C], f32r)
            nc.gpsimd.dma_start(starts_r[:], starts.bitcast(f32r))
            ends_r = cp.tile([1, C], f32r)
            nc.gpsimd.dma_start(ends_r[:], ends.bitcast(f32r))
            iota_sb = cp.tile([128, TCL], f32)
            nc.gpsimd.dma_start(iota_sb[:], iot)
            ssel_sb = cp.tile([NH, H], f32r)
            nc.gpsimd.dma_start(ssel_sb[:], ssel.bitcast(f32r))
            ones1w = cp.tile([1, 512], bf16)     # warmup rhs
            nc.vector.memset(ones1w, 1.0)
            ones1b = cp.tile([1, 128], bf16)
            nc.gpsimd.dma_start(ones1b[:], onesb.unsqueeze(0))
            bvl_sb = cp.tile([1, VW], bf16)
            nc.gpsimd.dma_start(bvl_sb[:], bvl)
            eps1 = cp.tile([1, 1], f32)
            nc.vector.memset(eps1, EPS)
            onescol = cp.tile([128, 1], f32r)    # partition-colsum lhsT
            nc.gpsimd.dma_start(onescol[:], onesv.unsqueeze(1).bitcast(f32r))
            gcol = cp.tile([128, HC], f32)      # gamma as per-partition cols
            nc.gpsimd.dma_start(gcol[:], gco.rearrange("(c p) -> p c", p=128))
            bcol = cp.tile([128, HC], f32)
            nc.gpsimd.dma_start(bcol[:], bco.rearrange("(c p) -> p c", p=128))
            b1ccol = cp.tile([128, HC], f32)
            nc.gpsimd.dma_start(b1ccol[:], b1c.rearrange("(c p) -> p c", p=128))
            b1col = cp.tile([128, IC], f32)
            nc.gpsimd.dma_start(b1col[:], b1r.rearrange("(c p) -> p c", p=128))
            b2ccol = cp.tile([128, HC], f32)
            nc.gpsimd.dma_start(b2ccol[:], b2c.rearrange("(c p) -> p c", p=128))
            eps2s = cp.tile([1, 1], f32)    # LN2 eps (scaled in fp8 mode)
            nc.vector.memset(eps2s, EPS * (S2 * S2 if fp8 else 1.0))

            # x1 kept (up to) three ways: matmul operand, unit-scale residual
            # basis, S2-scaled residual for the (scaled-W2) FFN2 add.
            x1b = x1p.tile([128, HC, C], bf16, name="x1b")
            if fp8:
                x1mm = x1p.tile([128, HC, C], mm_dt, name="x1mm")
                x1s2 = x1p.tile([128, HC, C], bf16, name="x1s2")
            else:
                x1mm, x1s2 = x1b, x1b

            # FFN weight tiles allocated early (stable addresses); all of W2
            # goes on the gpsimd queue (idle during attention), W1 groups 0-2
            # trail the attention-critical loads on sync.
            if fp8:
                w1g = [w1p.tile([128, HC // 2, 2, WG * 128], fp8e4, tag="w1",
                                name=f"w1g{g}") for g in range(3)]
                w2g = [w2p.tile([128, WG // 2, 2, H], fp8e4, tag="w2",
                                name=f"w2g{g}") for g in range(NG)]
            else:
                w1g = [w1p.tile([128, HC, WG * 128], bf16, tag="w1",
                                name=f"w1g{g}") for g in range(3)]
                w2g = [w2p.tile([128, WG, H], bf16, tag="w2",
                                name=f"w2g{g}") for g in range(NG)]

            # ---------------- attention (feature-major) ----------------
            with (
                tc.tile_pool(name="attn", bufs=1) as ap_,
                tc.tile_pool(name="attn_s", bufs=2) as asml,
            ):
                xTc = [ap_.tile([128, TW], bf16, name=f"xTc{c}")
                       for c in range(HC)]
                wvlc = [ap_.tile([128, VW], bf16, name=f"wvlc{c}")
                        for c in range(HC)]
                xTr = xT.rearrange("(c p) t -> c p t", p=128)
                wvlr = wvl.rearrange("(c p) n -> c p n", p=128)
                # attention-critical loads first, round-robin on two queues
                qs = [nc.sync, nc.scalar]
                for c in range(HC):
                    qs[0].dma_start(wvlc[c][:], wvlr[c])
                    qs[1].dma_start(xTc[c][:], xTr[c])

                # separate tiles for e*V and e: keeps the Z matmul (reads e)
                # from false-serializing behind the e*V DVE multiply (Tile
                # tracks deps per tile)
                veV = ap_.tile([128, TCL, H], bf16)   # e*V, token-major
                veE = ap_.tile([128, TCL, NH], bf16)  # e,   token-major
                wt = ap_.tile([128, TCL, C], bf16)    # W[t, s] 0/1 window
                ctxN = ap_.tile([128, HC, C], bf16)   # normalized ctx
                ycs = ap_.tile([128, HC, C], bf16)    # out_proj + b1c

                with (
                    tc.tile_pool(name="psAux", bufs=1, space="PSUM") as psAux,
                    tc.tile_pool(name="psV", bufs=2, space="PSUM") as psV,
                ):
                    # warm up the PE (HAM clock gate) while the loads land;
                    # also preload the Exp activation table.
                    wf = asml.tile([1, 128], bf16, tag="wf")
                    nc.vector.memset(wf, 1.0)
                    dexp = asml.tile([1, 1], f32, tag="dexp")
                    nc.scalar.activation(dexp[:], eps1[:], Act.Exp)
                    dum = psAux.tile([128, 512], f32, tag="warm")
                    for k in range(8):
                        nc.tensor.matmul(dum[:], wf[:], ones1w[:],
                                         start=(k == 0), stop=(k == 7))
                    # broadcast span starts/ends across partitions on the PE
                    startsB = psAux.tile([128, C], f32, tag="sb",
                                         name="startsB")
                    nc.tensor.matmul(startsB[:], ones1[:], starts_r[:],
                                     start=True, stop=True)
                    endsB = psAux.tile([128, C], f32, tag="eb",
                                       name="endsB")
                    nc.tensor.matmul(endsB[:], ones1[:], ends_r[:],
                                     start=True, stop=True)
                    # remaining weights on the two HWDGE queues (keeping the
                    # gpsimd queue empty so PSUM pool releases aren't stuck
                    # behind slow SWDGE DMA waits)
                    wout_sb = ap_.tile([128, HC, H], bf16)
                    nc.sync.dma_start(wout_sb[:],
                                      wout.rearrange("(c p) n -> p c n", p=128))
                    for g in range(3):
                        nc.sync.dma_start(w1g[g][:], w1t[g])
                    for g in range(NG):
                        qs[g % 2].dma_start(w2g[g][:], w2t[g])

                    # W[t, s] = (start_s <= t) & (t < end_s)  (needs no x)
                    for t in range(TCL):
                        lt = asml.tile([128, C], f32, tag="uexp", name="lt")
                        nc.vector.tensor_scalar(
                            lt[:], endsB[:], iota_sb[:, t:t + 1], None,
                            Op.is_gt)
                        nc.vector.scalar_tensor_tensor(
                            wt[:, t, :], startsB[:], iota_sb[:, t:t + 1],
                            lt[:], Op.is_le, Op.mult)

                    # V' = x @ [Wv.T | wq2.T] (+ e-logit bias)  (token-major)
                    for t in range(TCL):
                        vp = psV.tile([128, VW], f32, tag="vp")
                        for lo, hi in ((0, 512), (512, VW)):
                            for c in range(HC):
                                nc.tensor.matmul(
                                    vp[:, lo:hi],
                                    xTc[c][:, t * 128:(t + 1) * 128],
                                    wvlc[c][:, lo:hi],
                                    start=(c == 0),
                                    stop=(c == HC - 1 and hi != VW),
                                )
                        nc.tensor.matmul(
                            vp[:, H:VW], ones1b[:], bvl_sb[:, H:VW],
                            start=False, stop=True,
                        )
                        # e = exp(logits)
                        nc.scalar.activation(veE[:, t, :], vp[:, H:VW],
                                             Act.Exp)
                        # veV[:, t] = V * e (per-head broadcast of e)
                        e_b = veE[:, t, :].unsqueeze(2).broadcast_to(
                            [128, NH, HD])
                        nc.vector.tensor_tensor(
                            veV[:, t, :].rearrange("p (n d) -> p n d", d=HD),
                            vp[:, 0:H].rearrange("p (n d) -> p n d", d=HD),
                            e_b, Op.mult,
                        )
                    # preload the Sqrt table now that Exp is done with ACT
                    # (read a ve value so this schedules AFTER the Exp ops)
                    dsq = asml.tile([1, 1], f32, tag="dexp", name="dsq")
                    nc.scalar.activation(dsq[:], veE[0:1, TCL - 1, 0:1],
                                         Act.Sqrt)

                with (
                    tc.tile_pool(name="psMM", bufs=6, space="PSUM") as psMM,
                    tc.tile_pool(name="psS", bufs=2, space="PSUM") as psS,
                ):
                    cps = [psMM.tile([128, C], f32, tag="mm512",
                                     name=f"cp{c}") for c in range(HC)]
                    # Z[n, s] then u = 1/Z  (Z >= exp(tiny logit) ~ 1 always:
                    # every span keeps >= 1 in-window token; approx_fast is
                    # ~18 bits, ample for an attn scale kept in bf16 later)
                    zp = psS.tile([NH, C], f32, tag="small", name="zp")
                    for t in range(TCL):
                        nc.tensor.matmul(
                            zp[:], veE[:, t, :], wt[:, t, :],
                            start=(t == 0), stop=(t == TCL - 1))
                    uf = asml.tile([NH, C], f32, tag="uf", name="uf")
                    nc.vector.reciprocal_approx_fast(uf[:], zp[:])
                    u_sb = asml.tile([NH, C], f32r, tag="u")
                    with nc.allow_low_precision(reason="f32r is fp32-width"):
                        nc.vector.tensor_copy(u_sb[:], uf[:])

                    # raw ctx accumulation + u broadcast/apply
                    uexps = []
                    for c in range(HC):
                        cp_ = cps[c]
                        for t in range(TCL):
                            nc.tensor.matmul(
                                cp_[:], veV[:, t, c * 128:(c + 1) * 128],
                                wt[:, t, :],
                                start=(t == 0), stop=(t == TCL - 1))
                        up = psS.tile([128, C], f32, tag="small",
                                      name=f"up{c}")
                        nc.tensor.matmul(
                            up[:], ssel_sb[:, c * 128:(c + 1) * 128],
                            u_sb[:], start=True, stop=True)
                        uexp = asml.tile([128, C], bf16, tag="uexp",
                                         name=f"uexp{c}")
                        nc.scalar.activation(uexp[:], up[:], Act.Identity)
                        uexps.append(uexp)
                        if c >= 1:
                            nc.vector.tensor_tensor(ctxN[:, c - 1, :],
                                                    cps[c - 1][:],
                                                    uexps[c - 1][:], Op.mult)
                    nc.vector.tensor_tensor(ctxN[:, HC - 1, :], cps[HC - 1][:],
                                            uexps[HC - 1][:], Op.mult)

                    # out_proj (centered weights) + LN1, feature-major
                    varp = psS.tile([1, C], f32, tag="small", name="varp")
                    sqs = []
                    for m in range(HC):
                        aop = psMM.tile([128, C], f32, tag="mm512",
                                        name=f"aop{m}")
                        for c in range(HC):
                            nc.tensor.matmul(
                                aop[:], wout_sb[:, c, m * 128:(m + 1) * 128],
                                ctxN[:, c, :],
                                start=(c == 0), stop=(c == HC - 1))
                        # yc = aop + centered bias (per-partition) on ACT
                        nc.scalar.activation(ycs[:, m, :], aop[:],
                                             Act.Identity,
                                             bias=b1ccol[:, m:m + 1])
                        sq = asml.tile([128, C], f32r, tag="sq",
                                       name=f"sq{m}")
                        with nc.allow_low_precision(reason="f32r fp32-width"):
                            nc.vector.tensor_tensor(sq[:], ycs[:, m, :],
                                                    ycs[:, m, :], Op.mult)
                        sqs.append(sq)
                        nc.tensor.matmul(
                            varp[:], onescol[:], sq[:],
                            start=(m == 0), stop=(m == HC - 1))
                    # keep the PE warm through the serial rstd chain (a >3.4us
                    # idle gap would re-throttle the PE clock to half rate);
                    # reading ycs[5] anchors these INTO the chain window --
                    # dependency-free matmuls would get scheduled early
                    wrm = psS.tile([128, C], f32, tag="small", name="wrm")
                    for k in range(10):
                        nc.tensor.matmul(wrm[:], ycs[:, HC - 1, 0:128],
                                         ycs[:, HC - 1, :],
                                         start=(k == 0), stop=(k == 9))
                    # rstd = 1/sqrt(varp/H + eps); approx_fast (~18 bits) is
                    # ample for a scale kept in bf16 downstream
                    sd = asml.tile([1, C], f32, tag="sd")
                    nc.scalar.activation(sd[:], varp[:], Act.Sqrt,
                                         bias=eps1[:], scale=1.0 / H)
                    rf = asml.tile([1, C], f32, tag="rf2", name="rf")
                    nc.vector.reciprocal_approx_fast(rf[:], sd[:])
                    rstd = asml.tile([1, C], bf16, tag="rstd")
                    with nc.allow_low_precision(reason="per-span scale; "
                                                "bf16 downstream"):
                        nc.vector.tensor_copy(rstd[:], rf[:])
                    rp = psS.tile([128, C], f32, tag="small", name="rp")
                    nc.tensor.matmul(rp[:], ones1b[:], rstd[:],
                                     start=True, stop=True)
                    # second warm group: bridges rp -> x1b[0] -> first FFN1 MM
                    # (anchored on rstd so it lands exactly in that window)
                    wrm_b = psS.tile([128, C], f32, tag="small", name="wrm_b")
                    for k in range(4):
                        nc.tensor.matmul(wrm_b[:], ones1b[:], rstd[:],
                                         start=(k == 0), stop=(k == 3))
                    rpb = asml.tile([128, C], bf16, tag="rpb")
                    nc.scalar.activation(rpb[:], rp[:], Act.Identity)
                    for m in range(HC):
                        if gb_identity:
                            # chunk 0 straight off the PSUM broadcast (rpb
                            # lags by one ACT op); rest via bf16 2x-mode
                            if m == 0:
                                nc.vector.tensor_tensor(x1b[:, m, :],
                                                        ycs[:, m, :],
                                                        rp[:], Op.mult)
                            else:
                                nc.vector.tensor_tensor(x1b[:, m, :],
                                                        ycs[:, m, :],
                                                        rpb[:], Op.mult)
                        else:
                            tmp = asml.tile([128, C], f32, tag="tmp",
                                            name="tmp")
                            nc.vector.tensor_tensor(tmp[:], ycs[:, m, :],
                                                    rp[:], Op.mult)
                            nc.scalar.activation(x1b[:, m, :], tmp[:],
                                                 Act.Identity,
                                                 scale=gcol[:, m:m + 1],
                                                 bias=bcol[:, m:m + 1])
                        if fp8:
                            # fp8 matmul operand (ACT) + S2-scaled residual
                            nc.scalar.activation(x1mm[:, m, :], x1b[:, m, :],
                                                 Act.Copy)
                            nc.vector.tensor_scalar(x1s2[:, m, :],
                                                    x1b[:, m, :], S2, None,
                                                    Op.mult)

            # ---------------- FFN + LN2 (feature-major, pipelined) --------
            with (
                tc.tile_pool(name="ffn", bufs=1) as fp_,
                tc.tile_pool(name="ffn_s", bufs=2) as fsml,
                tc.tile_pool(name="ffn_o", bufs=6) as fout,
                tc.tile_pool(name="psH", bufs=2, space="PSUM") as psH,
                tc.tile_pool(name="psF", bufs=1, space="PSUM") as psF,
            ):
                h1s = fp_.tile([128, IC, C], mm_dt)
                zF = fp_.tile([128, HC, C], bf16)
                h2T = [psF.tile([128, C], f32, tag=f"h2_{m}", name=f"h2T{m}")
                       for m in range(HC)]

                x1op = x1mm if fp8 else x1b

                def ffn2_pair(ip):
                    g, lp = divmod(ip, WG // 2)
                    for m in range(HC):
                        nc.tensor.matmul(
                            h2T[m][:],
                            w2g[g][:, lp, :, m * 128:(m + 1) * 128],
                            h1s[:, 2 * ip:2 * ip + 2, :],
                            start=(ip == 0), stop=False, perf_mode=DR)

                def ffn2_single(i):
                    g, l = divmod(i, WG)
                    for m in range(HC):
                        nc.tensor.matmul(
                            h2T[m][:],
                            w2g[g][:, l, m * 128:(m + 1) * 128],
                            h1s[:, i, :],
                            start=(i == 0), stop=False)

                NTAIL = 7        # bf16: last i-chunks run m-outer
                NTAILP = 4       # fp8: last i-chunk pairs run m-outer

                for i in range(IC):
                    g, l = divmod(i, WG)
                    if l == 0 and g >= 3:  # stream remaining W1 groups
                        if fp8:
                            w1g[g % 3] = w1p.tile(
                                [128, HC // 2, 2, WG * 128], fp8e4,
                                tag="w1", name=f"w1g{g}")
                        else:
                            w1g[g % 3] = w1p.tile(
                                [128, HC, WG * 128], bf16,
                                tag="w1", name=f"w1g{g}")
                        nc.sync.dma_start(w1g[g % 3][:], w1t[g])
                    w1 = w1g[g % 3]
                    h1p = psH.tile([128, C], f32, tag="h1p")
                    if fp8:
                        for cp2 in range(HC // 2):
                            nc.tensor.matmul(
                                h1p[:],
                                w1[:, cp2, :, l * 128:(l + 1) * 128],
                                x1op[:, 2 * cp2:2 * cp2 + 2, :],
                                start=(cp2 == 0), stop=(cp2 == HC // 2 - 1),
                                perf_mode=DR)
                    else:
                        for c in range(HC):
                            nc.tensor.matmul(
                                h1p[:], w1[:, c, l * 128:(l + 1) * 128],
                                x1op[:, c, :],
                                start=(c == 0), stop=(c == HC - 1))
                    nc.scalar.activation(h1s[:, i, :], h1p[:], Act.Relu,
                                         bias=b1col[:, i:i + 1],
                                         scale=1.0 / S1 if fp8 else 1.0)
                    if fp8:
                        if i % 2 == 1 and 1 <= i // 2 <= NP - NTAILP:
                            ffn2_pair(i // 2 - 1)
                    else:
                        if 1 <= i <= IC - NTAIL:
                            ffn2_single(i - 1)

                # LN2 (no mean: W2/b2 centered, x1 zero-mean) + output.
                # Last chunks run m-outer: h2T[m] finishes early so its
                # z/square/var chain overlaps the remaining matmuls.
                varp2 = psH.tile([1, C], f32, tag="h1p", name="varp2")
                sq2s = []
                for m in range(HC):
                    if fp8:
                        for ip in range(NP - NTAILP, NP):
                            g, lp = divmod(ip, WG // 2)
                            nc.tensor.matmul(
                                h2T[m][:],
                                w2g[g][:, lp, :, m * 128:(m + 1) * 128],
                                h1s[:, 2 * ip:2 * ip + 2, :],
                                start=False, stop=(ip == NP - 1),
                                perf_mode=DR)
                    else:
                        for i in range(IC - NTAIL, IC):
                            g, l = divmod(i, WG)
                            nc.tensor.matmul(
                                h2T[m][:],
                                w2g[g][:, l, m * 128:(m + 1) * 128],
                                h1s[:, i, :],
                                start=False, stop=(i == IC - 1))
                    # z = (h2 + S2*b2c) + S2*x1  (one DVE op, feature-major)
                    nc.vector.scalar_tensor_tensor(
                        zF[:, m, :], h2T[m][:], b2ccol[:, m:m + 1],
                        x1s2[:, m, :], Op.add, Op.add)
                    sq2 = fsml.tile([128, C], f32r, tag="sq2", name=f"sq2{m}")
                    with nc.allow_low_precision(reason="f32r is fp32-width"):
                        nc.scalar.activation(sq2[:], zF[:, m, :], Act.Square)
                    sq2s.append(sq2)
                    if m >= 2:
                        nc.tensor.matmul(
                            varp2[:], onescol[:], sq2s[m - 2][:],
                            start=(m - 2 == 0), stop=False)
                for m in (HC - 2, HC - 1):
                    nc.tensor.matmul(
                        varp2[:], onescol[:], sq2s[m][:],
                        start=False, stop=(m == HC - 1))
                wrm2 = psH.tile([128, C], f32, tag="h1p", name="wrm2")
                for k in range(10):
                    nc.tensor.matmul(wrm2[:], zF[:, HC - 1, 0:128],
                                     zF[:, HC - 1, :],
                                     start=(k == 0), stop=(k == 9))
                sd2 = fsml.tile([1, C], f32, tag="sd2")
                nc.scalar.activation(sd2[:], varp2[:], Act.Sqrt,
                                     bias=eps2s[:], scale=1.0 / H)
                rf2 = fsml.tile([1, C], f32, tag="rf2b", name="rf2")
                nc.vector.reciprocal_approx_fast(rf2[:], sd2[:])
                rstd2 = fsml.tile([1, C], bf16, tag="rstd2")
                with nc.allow_low_precision(reason="output is bf16 anyway"):
                    nc.vector.tensor_copy(rstd2[:], rf2[:])
                rp2 = psH.tile([128, C], f32, tag="h1p", name="rp2")
                nc.tensor.matmul(rp2[:], ones1b[:], rstd2[:],
                                 start=True, stop=True)
                rp2b = fsml.tile([128, C], bf16, tag="rp2b")
                nc.scalar.activation(rp2b[:], rp2[:], Act.Identity)
                oqs = [nc.sync, nc.scalar, nc.gpsimd]
                for m in range(HC):
                    o = fout.tile([128, C], bf16, tag="o", name=f"o{m}")
                    if gb_identity:
                        # chunk 0 straight off the PSUM broadcast (rp2b lags
                        # by one ACT op); rest via bf16 2x-mode
                        nc.vector.tensor_tensor(o[:], zF[:, m, :],
                                                rp2[:] if m == 0 else rp2b[:],
                                                Op.mult)
                    else:
                        on = fout.tile([128, C], f32, tag="on", name=f"on{m}")
                        nc.vector.tensor_tensor(on[:], zF[:, m, :],
                                                rp2b[:], Op.mult)
                        nc.scalar.activation(o[:], on[:], Act.Identity,
                                             scale=gcol[:, m:m + 1],
                                             bias=bcol[:, m:m + 1])
                    oqs[m % 2].dma_start(out[m * 128:(m + 1) * 128, :], o[:])
    nc.compile()
    return nc


def _plan(span_masks, span_ids):
    """Compact unmasked spans, dedup identical (start,end) pairs, sort by
    start, split each batch's unique spans across its 4 cores (so each core
    covers a narrow token window). Returns (per_core_idx, C, scatter) where
    per_core_idx holds representative original span indices and scatter maps
    every unmasked original span to (its batch's unique-span position)."""
    masks = np.asarray(span_masks)
    sid = np.asarray(span_ids)
    per_core_idx, scatter = [], []
    cpb = NCORES // B
    for b in range(B):
        idx = np.nonzero(masks[b])[0]
        key = sid[b, idx, 0] * 1024 + sid[b, idx, 1]   # start-major sort key
        uk, first, inv = np.unique(key, return_index=True,
                                   return_inverse=True)
        rep = idx[first]                               # one original per uniq
        per_core_idx.extend(np.array_split(rep, cpb))
        scatter.append((idx, inv))
    cmax = max(len(ix) for ix in per_core_idx)
    C = (cmax + 7) // 8 * 8
    C = min(max(C, 128), 512)
    return per_core_idx, C, scatter


def _host_prepare(inputs, per_core_idx, C):
    """Host-side packing: tiny index/weight reshapes, no heavy math.
    Returns (in_maps, TCL, sorted per_core_idx)."""
    import ml_dtypes
    bf = ml_dtypes.bfloat16
    f8 = ml_dtypes.float8_e4m3

    tr = np.asarray(inputs["token_reps"], dtype=np.float32)
    span_ids = np.asarray(inputs["span_ids"]).astype(np.int64)
    pe = np.asarray(inputs["pe"], dtype=np.float32)
    q0 = np.asarray(inputs["dummy_query"], dtype=np.float32)
    in_w = np.asarray(inputs["in_proj_w"], dtype=np.float32)
    in_b = np.asarray(inputs["in_proj_b"], dtype=np.float32)
    wo = np.asarray(inputs["out_proj_w"], dtype=np.float32)
    bo = np.asarray(inputs["out_proj_b"], dtype=np.float32)
    g = np.asarray(inputs["norm_g"], dtype=np.float32)
    bb = np.asarray(inputs["norm_b"], dtype=np.float32)
    w1 = np.asarray(inputs["ffn_w1"], dtype=np.float32)
    b1 = np.asarray(inputs["ffn_b1"], dtype=np.float32)
    w2 = np.asarray(inputs["ffn_w2"], dtype=np.float32)
    b2 = np.asarray(inputs["ffn_b2"], dtype=np.float32)

    Wq, Wk, Wv = in_w[0:H], in_w[H:2 * H], in_w[2 * H:3 * H]
    bq, bk, bv = in_b[0:H], in_b[H:2 * H], in_b[2 * H:3 * H]

    q = q0 @ Wq.T + bq
    qs = (q / np.sqrt(HD)).astype(np.float32)
    wq2 = np.stack([qs[n * HD:(n + 1) * HD] @ Wk[n * HD:(n + 1) * HD]
                    for n in range(NH)])                      # (12, 768)
    constv = np.array([qs[n * HD:(n + 1) * HD] @ bk[n * HD:(n + 1) * HD]
                       for n in range(NH)], dtype=np.float32)

    wvl = np.concatenate([Wv.T, wq2.T], axis=1).astype(np.float32)   # (768, 780)
    bvl = np.concatenate([np.zeros(H, np.float32), constv])[None, :]

    wout_c = wo - wo.mean(axis=0, keepdims=True)
    wout_ct = np.ascontiguousarray(wout_c.T).astype(np.float32)      # (768, 768)
    # fold V-bias through the (centered) out-projection into the LN1 bias
    b1c_full = bo + q0 + bv @ wout_c.T
    b1c = (b1c_full - b1c_full.mean()).astype(np.float32)

    ssel = np.zeros((NH, H), dtype=np.float32)
    for n in range(NH):
        ssel[n, n * HD:(n + 1) * HD] = 1.0

    # packed FFN weights:
    w2ct = w2.T - w2.T.mean(axis=1, keepdims=True)     # (INTERMED, H)
    if FP8:
        w1T = (w1.T * S1).reshape(HC // 2, 2, 128, NG, WG * 128)
        w1tp = np.ascontiguousarray(
            w1T.transpose(3, 2, 0, 1, 4)).astype(f8)   # (g, p, c2, two, 512)
        b2c = ((b2 - b2.mean()) * S2).astype(np.float32)
        w2T = (w2ct * S2).reshape(NG, WG // 2, 2, 128, H)
        w2tp = np.ascontiguousarray(
            w2T.transpose(0, 3, 1, 2, 4)).astype(f8)   # (g, p, l2, two, h)
    else:
        w1T = w1.T.reshape(HC, 128, NG, WG * 128)      # (c, p, g, l*128+n)
        w1tp = np.ascontiguousarray(
            w1T.transpose(2, 1, 0, 3)).astype(bf)      # (g, p, c, 512)
        b2c = (b2 - b2.mean()).astype(np.float32)
        w2T = w2ct.reshape(NG, WG, 128, H)             # (g, l, p, h)
        w2tp = np.ascontiguousarray(
            w2T.transpose(0, 2, 1, 3)).astype(bf)      # (g, p, l, h)

    x = tr + pe[None, :T]                              # (B, T, H)
    xTs = [np.ascontiguousarray(x[b].T).astype(bf) for b in range(B)]

    starts_all = span_ids[..., 0].astype(np.float32)   # (B, S)
    lens_all = (span_ids[..., 1] - span_ids[..., 0]).astype(np.float32)
    ends_all = starts_all + lens_all

    # per-core token windows (spans already sorted by start)
    sorted_idx, t0s, wmax = [], [], 1
    for core in range(NCORES):
        b = core // (NCORES // B)
        idx = per_core_idx[core]
        sorted_idx.append(idx)
        if len(idx):
            t0 = int(starts_all[b, idx].min())
            t1 = int(ends_all[b, idx].max())
        else:
            t0, t1 = 0, 1
        t0s.append(t0)
        wmax = max(wmax, t1 - t0)
    TCL = min((wmax + 127) // 128, (T + 127) // 128)
    TW = TCL * 128

    shared = dict(wvl=wvl.astype(bf), bvl=bvl.astype(bf),
                  ssel=ssel, wout=wout_ct.astype(bf),
                  b1c=b1c, gco=g, bco=bb, w1t=w1tp, b1r=b1, w2t=w2tp, b2c=b2c,
                  onesv=np.ones(128, dtype=np.float32),
                  onesb=np.ones(128, dtype=bf))
    in_maps = []
    for core in range(NCORES):
        b = core // (NCORES // B)
        idx = sorted_idx[core]
        t0 = t0s[core]
        st = np.full(C, float(t0), np.float32)
        en = np.full(C, float(t0 + 1), np.float32)  # pad spans: 1 token
        st[:len(idx)] = starts_all[b, idx]
        en[:len(idx)] = ends_all[b, idx]
        xw = np.zeros((H, TW), dtype=bf)
        hi = min(T, t0 + TW)
        xw[:, :hi - t0] = xTs[b][:, t0:hi]
        iota = (float(t0) + np.arange(128, dtype=np.float32)[:, None]
                + 128.0 * np.arange(TCL, dtype=np.float32)[None, :])
        m = dict(shared)
        m["xT"] = xw
        m["iota"] = np.ascontiguousarray(iota)
        m["starts"] = np.ascontiguousarray(st[None, :])
        m["ends"] = np.ascontiguousarray(en[None, :])
        in_maps.append(m)
    return in_maps, TCL, sorted_idx


def kernel(**inputs) -> np.ndarray:
    global _COMPILED
    from concourse.bass_utils import run_bass_kernel_spmd

    per_core_idx, C, scatter = _plan(inputs["span_masks"], inputs["span_ids"])
    gbi = (np.allclose(np.asarray(inputs["norm_g"], dtype=np.float32), 1.0)
           and np.allclose(np.asarray(inputs["norm_b"], dtype=np.float32), 0.0))
    in_maps, TCL, sorted_idx = _host_prepare(inputs, per_core_idx, C)
    key = (C, TCL, gbi, FP8)
    if key not in _COMPILED:
        _COMPILED[key] = _build(C, TCL, gb_identity=gbi, fp8=FP8)
    nc = _COMPILED[key]
    res = run_bass_kernel_spmd(nc, in_maps, core_ids=list(range(NCORES)))
    full = np.zeros((B, S, H), dtype=np.float32)
    cpb = NCORES // B
    for b in range(B):
        # concat the batch's unique-span outputs in (sorted) unique order,
        # then fan out to all duplicate original spans
        outs = []
        for k in range(cpb):
            core = b * cpb + k
            n = len(sorted_idx[core])
            o = np.asarray(res.results[core]["out"]).astype(np.float32)
            outs.append(o[:, :n].T)                    # [n, H]
        uniq_out = np.concatenate(outs, axis=0)
        idx, inv = scatter[b]
        full[b, idx] = uniq_out[inv]
    return full
